# revision 4
# baseline (speedup 1.0000x reference)
"""Causal multi-head attention on 8 Trainium2 NeuronCores — v2.

Sharding: core c -> (batch g = c // 4, head-group p = c % 4, heads 4p..4p+3).
All matmuls bf16 (f32 PSUM accumulation). Causal work tiled at 128-row
k-tile granularity (fully-masked tiles skipped). Per k-tile the scores land
transposed [kpos, q] in PSUM, exp runs on the scalar engine into bf16
probs, and AV accumulates ctx^T[dk+1, q] per head (one PSUM bank per head:
a bank must never hold two open accumulation groups) with a ones column on
V putting the softmax denominators in partition 64; normalization is a DVE
reciprocal + gpsimd partition_broadcast + DVE multiply into the oproj
stationary layout. V is projected into natural [kpos, feat] layout from
resident xv. Output-projection partials are copied to bf16 SBUF, DMA'd to
DRAM per 512-row wave, ReduceScattered over each 4-core batch group, and
b_o is added on host. A budgeted interleaver paces the scores stream to
the activation engine's exp rate and fills the in-order PE with V/AV/oproj
units so no engine stalls the others.
"""

import os as os_mod
import numpy as np

B, S, D, H = 2, 2048, 1024, 16
DK = D // H
N_CORES = 8
FPC = 256  # features (head dims) per core

_CACHE = {}


def _build_nc():
    import concourse.mybir as mybir
    import concourse.tile as tile
    from concourse import bacc

    F32 = mybir.dt.float32
    BF16 = mybir.dt.bfloat16
    Exp = mybir.ActivationFunctionType.Exp

    nc = bacc.Bacc("TRN2", target_bir_lowering=False, debug=False, num_devices=8)

    cA = nc.dram_tensor("cA", [128, 2048], BF16, kind="ExternalInput")  # wq
    cB1 = nc.dram_tensor("cB1", [128, 2048], BF16, kind="ExternalInput")  # wk
    cB2 = nc.dram_tensor("cB2", [128, 4736], BF16, kind="ExternalInput")  # wv|wo|mask4|ident
    cF = nc.dram_tensor("cF", [128, 260], F32, kind="ExternalInput")  # bq|bk|bvt
    xq = nc.dram_tensor("xq", [D, S], BF16, kind="ExternalInput")
    xk = nc.dram_tensor("xk", [D, S], BF16, kind="ExternalInput")
    xv = nc.dram_tensor("xv", [D, S], BF16, kind="ExternalInput")
    out = nc.dram_tensor("out", [512, D], BF16, kind="ExternalOutput")

    no_rs = bool(os_mod.environ.get("BASS_SIM_NO_RS"))

    with tile.TileContext(nc) as tc:
        with (
            tc.tile_pool(name="consts", bufs=1) as consts,
            tc.tile_pool(name="persist", bufs=1) as persist,
            tc.tile_pool(name="xin", bufs=6) as xin,
            tc.tile_pool(name="prp", bufs=1) as prp,
            tc.tile_pool(name="small", bufs=3) as small,
            tc.tile_pool(name="oout", bufs=6) as oout,
            tc.tile_pool(name="dram", bufs=1, space="DRAM") as dram,
        ):
            # ---------------- constants ----------------
            cA_s = consts.tile([128, 2048], BF16, tag="cA", name="cA_s")
            cB1_s = consts.tile([128, 2048], BF16, tag="cB1", name="cB1_s")
            cB2_s = consts.tile([128, 4736], BF16, tag="cB2", name="cB2_s")
            cF_s = consts.tile([128, 260], F32, tag="cF", name="cF_s")
            wq_s = cA_s[:].rearrange("p (kc f) -> p kc f", kc=8)
            wk_s = cB1_s[:].rearrange("p (kc f) -> p kc f", kc=8)
            wv_s = cB2_s[:, 0:2048].rearrange("p (kc f) -> p kc f", kc=8)
            wo_s = cB2_s[:, 2048:4096].rearrange("p (c d) -> p c d", c=2)
            mask4_s = cB2_s[:, 4096:4608].rearrange("p (h x) -> p h x", h=4)
            ident_s = cB2_s[:, 4608:4736]
            bq_s = cF_s[:, 0:2]
            bk_s = cF_s[:, 2:4]
            bvt_s = cF_s[:, 4:260].rearrange("p (h x) -> p h x", h=4)

            # ---------------- persistent activations ----------------
            qT_s = persist.tile([128, 2, S], BF16, tag="qT", name="qT_s")
            kT_s = persist.tile([128, 2, S], BF16, tag="kT", name="kT_s")
            xv_s = persist.tile([128, 8, S], BF16, tag="xv", name="xv_s")
            v_s = persist.tile([128, 16, 4, 65], BF16, tag="v", name="v_s")
            ctxT_s = persist.tile([128, 2, S], BF16, tag="ctxT", name="ctxT_s")

            rs_in = [dram.tile([S // 2, D], BF16, name=f"rs_in{i}") for i in range(2)]
            rs_out = [dram.tile([256, D], BF16, name=f"rs_out{i}") for i in range(2)]

            # ones columns for the softmax denominators
            for h in range(4):
                nc.gpsimd.memset(v_s[:, :, h, 64:65], 1.0)

            # ---------------- input DMA stream (SP queue order) ----------------
            xq_t, xk_t = [], []
            nc.sync.dma_start(cA_s[:, 0:256], cA[:, 0:256])
            t = xin.tile([128, S], BF16, tag="x", name="xq0")
            nc.sync.dma_start(t[:], xq[0:128, :])
            xq_t.append(t)
            nc.sync.dma_start(cA_s[:, 256:2048], cA[:, 256:2048])
            for kc in range(1, 8):
                t = xin.tile([128, S], BF16, tag="x", name=f"xq{kc}")
                nc.sync.dma_start(t[:], xq[128 * kc : 128 * (kc + 1), :])
                xq_t.append(t)
            nc.sync.dma_start(cB1_s[:], cB1.ap())
            nc.sync.dma_start(cF_s[:], cF.ap())
            for kc in range(8):
                t = xin.tile([128, S], BF16, tag="x", name=f"xk{kc}")
                nc.sync.dma_start(t[:], xk[128 * kc : 128 * (kc + 1), :])
                xk_t.append(t)
            nc.sync.dma_start(cB2_s[:], cB2.ap())
            for kc in range(8):
                nc.sync.dma_start(xv_s[:, kc, :], xv[128 * kc : 128 * (kc + 1), :])

            # ---------------- phase 1: Q/K projections ----------------
            def proj_pass(x_t, w_s, b_s, outT, psP, split_adds=False):
                ps = {}
                for pt in range(2):
                    for qb in range(4):
                        ps[(pt, qb)] = psP.tile(
                            [128, 512], F32, tag="pp", name=f"ps{pt}{qb}"
                        )
                for kc in range(7):
                    for pt in range(2):
                        for qb in range(4):
                            nc.tensor.matmul(
                                ps[(pt, qb)][:],
                                w_s[:, kc, 128 * pt : 128 * (pt + 1)],
                                x_t[kc][:, 512 * qb : 512 * (qb + 1)],
                                start=(kc == 0),
                                stop=False,
                            )
                # final contraction step: emit the bias-add right after each
                # accumulator stops so the adds pipeline with the sweep
                for i, (pt, qb) in enumerate(
                    [(pt, qb) for pt in range(2) for qb in range(4)]
                ):
                    nc.tensor.matmul(
                        ps[(pt, qb)][:],
                        w_s[:, 7, 128 * pt : 128 * (pt + 1)],
                        x_t[7][:, 512 * qb : 512 * (qb + 1)],
                        start=False,
                        stop=True,
                    )
                    nc.vector.tensor_scalar_add(
                        outT[:, pt, 512 * qb : 512 * (qb + 1)],
                        ps[(pt, qb)][:],
                        b_s[:, pt : pt + 1],
                    )

            with tc.tile_pool(name="psP", bufs=8, space="PSUM") as psP:
                proj_pass(xq_t, wq_s, bq_s, qT_s, psP)
                proj_pass(xk_t, wk_s, bk_s, kT_s, psP, split_adds=True)

            # ---------------- phase 2: attention ----------------
            pr_t = {}  # (w, ki) -> probs tile [128, 4, 512] bf16 (wave q coords)
            prm_t = {}  # (w, diag ki) -> masked probs [128, 4, 128] bf16
            ctx_t = {}  # (w, qt) -> ctx psum tile [128, 4, 128] f32
            ctxn_t = {}  # (w, qt) -> normalized ctx sbuf [128, 4, 64] bf16

            psS_pool = [None]
            psC_pool = [None]

            def sc_unit(w, ki):
                """scores + exp (+ diag mask) for (wave, ktile), one
                head-pair (2-bank) PSUM tile per exp so PE rarely waits."""
                qoff = 128 * max(ki - 4 * w, 0)
                wdt = 512 - qoff
                pr = prp.tile(
                    [128, 4, 512], BF16, tag="pr", bufs=19, name=f"pr_{w}_{ki}"
                )
                pr_t[(w, ki)] = pr
                for hp in range(2):
                    sc = psS_pool[0].tile([128, 2, 512], F32, tag="sc", name="sc")
                    for j in range(2):
                        h = 2 * hp + j
                        r, pt = 64 * (h % 2), h // 2
                        nc.tensor.matmul(
                            sc[:, j, 0:wdt],
                            kT_s[r : r + 64, pt, 128 * ki : 128 * (ki + 1)],
                            qT_s[r : r + 64, pt, 512 * w + qoff : 512 * (w + 1)],
                            start=True,
                            stop=True,
                        )
                    nc.scalar.activation(
                        out=pr[:, 2 * hp : 2 * hp + 2, qoff:512],
                        in_=sc[:, :, 0:wdt],
                        func=Exp,
                        scale=0.125,
                    )
                if ki >= 4 * w:  # diag ktile: mask the upper triangle in place
                    nc.vector.tensor_mul(
                        pr[:, :, qoff : qoff + 128],
                        pr[:, :, qoff : qoff + 128],
                        mask4_s,
                    )

            def v_unit(st):
                pv = psV_pool[0].tile([128, 256], F32, tag="pv", name="pv")
                for kc in range(8):
                    nc.tensor.matmul(
                        pv[:],
                        xv_s[:, kc, 128 * st : 128 * (st + 1)],
                        wv_s[:, kc, :],
                        start=(kc == 0),
                        stop=(kc == 7),
                    )
                nc.vector.tensor_add(
                    v_s[:, st, :, 0:64],
                    pv[:].rearrange("p (h x) -> p h x", x=64),
                    bvt_s,
                )

            def av_open(w):
                # one [65, 512] strip per PSUM bank: a bank must never hold
                # more than one open accumulation group (start=True on one
                # strip invalidates other strips' pending sums in the bank)
                for h in range(4):
                    ctx_t[(w, h)] = psC_pool[0].tile(
                        [65, 512], F32, tag="ctx", name=f"ctx{w}{h}"
                    )

            def av_mm(w, ki):
                """ctx^T[dk+1, q] += V_aug_h^T @ probs per head: the ones
                column of V_aug puts the softmax denominators in partition
                64."""
                qoff = 128 * max(ki - 4 * w, 0)
                last = 4 * w + 3
                for h in range(4):
                    nc.tensor.matmul(
                        ctx_t[(w, h)][:, qoff:512],
                        v_s[:, ki, h, :],
                        pr_t[(w, ki)][:, h, qoff:512],
                        start=(ki == 0),
                        stop=(ki == last),
                        skip_group_check=True,
                    )

            def av_fin(w, h):
                """normalize ctx^T by the denominators in partition 64."""
                r, pt = 64 * (h % 2), h // 2
                ctx = ctx_t[(w, h)]
                recip = small.tile([1, 512], F32, tag="recip", name="recip")
                nc.vector.reciprocal(recip[:], ctx[64:65, :])
                rbc = small.tile([64, 512], F32, tag="rbc", name="rbc")
                nc.gpsimd.partition_broadcast(rbc[:], recip[:])
                nc.vector.tensor_mul(
                    ctxT_s[r : r + 64, pt, 512 * w : 512 * (w + 1)],
                    ctx[0:64, :],
                    rbc[:],
                )

            def po_unit(w, u):
                qt, nb = u // 2, u % 2
                st = 4 * w + qt
                half = w // 2
                po = psC_pool[0].tile([128, 512], F32, tag="ctx", name="po")
                for fc in range(2):
                    nc.tensor.matmul(
                        po[:],
                        ctxT_s[:, fc, 128 * st : 128 * (st + 1)],
                        wo_s[:, fc, 512 * nb : 512 * (nb + 1)],
                        start=(fc == 0),
                        stop=(fc == 1),
                    )
                ot = oout.tile([128, 512], BF16, tag="ot", name="ot")
                if w == 3:  # tail wave: Act is done with exps by then
                    nc.scalar.activation(
                        out=ot[:], in_=po[:],
                        func=mybir.ActivationFunctionType.Copy,
                    )
                else:
                    nc.vector.tensor_copy(ot[:], po[:])
                nc.sync.dma_start(
                    rs_in[half][
                        512 * (w % 2) + 128 * qt : 512 * (w % 2) + 128 * (qt + 1),
                        512 * nb : 512 * (nb + 1),
                    ],
                    ot[:],
                )

            def rs_sim_out(half):
                # sim-mode stand-in for the RS result copy; reads only the
                # first wave's first two qtiles of rs_in[half]
                if no_rs:
                    nc.sync.dma_start(
                        out[256 * half : 256 * (half + 1), :], rs_in[half][0:256, :]
                    )

            def rs_unit(half):
                if not no_rs:
                    import concourse.mybir as mybir_mod

                    nc.gpsimd.collective_compute(
                        "ReduceScatter",
                        mybir_mod.AluOpType.add,
                        replica_groups=[[0, 1, 2, 3], [4, 5, 6, 7]],
                        ins=[rs_in[half].opt()],
                        outs=[rs_out[half].opt()],
                    )
                    nc.sync.dma_start(
                        out[256 * half : 256 * (half + 1), :], rs_out[half][:]
                    )

            # ---- budgeted interleave: scores paced by Act; filler units
            # (V proj, AV, oproj) sized to keep the PE exactly as busy as
            # Act's per-ktile pace, in dependency (= PSUM rotation) order.
            WAVE_ORDER = (0, 1, 2, 3)
            sc_list = [(w, ki) for w in WAVE_ORDER for ki in range(4 * w + 4)]
            sc_idx = {u: i for i, u in enumerate(sc_list)}

            def wdt_of(w, ki):
                return 512 - 128 * max(ki - 4 * w, 0)

            def act_cost(w, ki):  # us of Act work per sc unit
                return (4 * wdt_of(w, ki) * 0.833 + 2 * 185) / 1000.0

            fillers = []  # (pe_cost_us, gate_sc_index, fn)

            def F(cost, gate, fn, *a):
                fillers.append((cost, gate, lambda a=a: fn(*a)))

            GV = 7  # V units gated until a few sc units in (xv lands late)
            for st in range(16):
                F(0.85, GV, v_unit, st)
            for w in WAVE_ORDER:
                F(0.0, None, av_open, w)
                for ki in range(4 * w + 4):
                    F(
                        4 * wdt_of(w, ki) * 0.4167 / 1000.0,
                        min(sc_idx[(w, ki)] + 3, len(sc_list)),
                        av_mm, w, ki,
                    )
                for h in range(4):
                    F(0.0, None, av_fin, w, h)
                for u in range(8):
                    F(0.43, None, po_unit, w, u)
                    if w in (1, 3) and u == 3:
                        F(0.0, None, rs_sim_out, w // 2)
                if w in (1, 3):
                    F(0.0, None, rs_unit, w // 2)

            state = {"budget": 0.0, "sci": 0, "popped": 0}

            def pump(force=False, max_pop=None):
                while fillers:
                    if max_pop is not None and state["popped"] >= max_pop:
                        break
                    cost, gate, fn = fillers[0]
                    if gate is not None and state["sci"] < gate:
                        break
                    if not force and state["budget"] < cost:
                        break
                    fillers.pop(0)
                    fn()
                    state["popped"] += 1
                    state["budget"] -= cost

            with tc.tile_pool(name="psS", bufs=2, space="PSUM") as psS:
                psS_pool[0] = psS
                with tc.tile_pool(name="psV", bufs=2, space="PSUM") as psV:
                    psV_pool = [psV]
                    # phase A: emit scores, pumping only the 16 V-proj fillers
                    while state["popped"] < 16:
                        w, ki = sc_list[state["sci"]]
                        sc_unit(w, ki)
                        state["sci"] += 1
                        state["budget"] += 1.10 * act_cost(w, ki) - 0.85
                        pump(max_pop=16)
                with tc.tile_pool(name="psC", bufs=4, space="PSUM") as psC:
                    psC_pool[0] = psC
                    while state["sci"] < len(sc_list):
                        w, ki = sc_list[state["sci"]]
                        sc_unit(w, ki)
                        state["sci"] += 1
                        state["budget"] += 1.10 * act_cost(w, ki) - 0.85
                        pump()
                    pump(force=True)

    nc.compile()
    return nc


def _prep_inputs(query, key_, value, w_q, b_q, w_k, b_k, w_v, b_v, w_o, b_o):
    """Build the 8 per-core input maps (host-side sharding / re-layout)."""
    import ml_dtypes

    f32 = np.float32
    bf16 = ml_dtypes.bfloat16

    def pack_w(wT_slice):  # [1024, 256] -> [128, 8, 256] -> [128, 2048]
        return (
            np.ascontiguousarray(
                wT_slice.reshape(8, 128, FPC).transpose(1, 0, 2).reshape(128, 2048)
            ).astype(bf16)
        )

    r = np.arange(128)
    mask = (r[None, :] >= r[:, None]).astype(f32)  # [kpos, q] allowed: q >= k
    mask4 = np.tile(mask, (1, 4)).astype(bf16)  # [128, 512]
    ident = np.eye(128, dtype=f32).astype(bf16)

    wqT = np.asarray(w_q, f32).T
    wkT = np.asarray(w_k, f32).T
    wvT = np.asarray(w_v, f32).T
    woT = np.asarray(w_o, f32).T

    xT = {}
    for g in range(B):
        xT[("q", g)] = np.ascontiguousarray(np.asarray(query[g], f32).T).astype(bf16)
        xT[("k", g)] = np.ascontiguousarray(np.asarray(key_[g], f32).T).astype(bf16)
        xT[("v", g)] = np.ascontiguousarray(np.asarray(value[g], f32).T).astype(bf16)

    in_maps = []
    for c in range(N_CORES):
        g, p = c // 4, c % 4
        fsel = slice(FPC * p, FPC * (p + 1))
        woc = (
            np.ascontiguousarray(
                woT[fsel, :].reshape(2, 128, D).transpose(1, 0, 2).reshape(128, 2048)
            ).astype(bf16)
        )
        cB2_arr = np.concatenate(
            [pack_w(wvT[:, fsel]), woc, mask4, ident], axis=1
        )
        bq_c = np.asarray(b_q, f32)[fsel].reshape(2, 128).T
        bk_c = np.asarray(b_k, f32)[fsel].reshape(2, 128).T
        bvt = np.broadcast_to(np.asarray(b_v, f32)[fsel], (128, FPC))
        cF_arr = np.concatenate([bq_c, bk_c, bvt], axis=1)
        in_maps.append(
            {
                "cA": pack_w(wqT[:, fsel]),
                "cB1": pack_w(wkT[:, fsel]),
                "cB2": np.ascontiguousarray(cB2_arr),
                "cF": np.ascontiguousarray(cF_arr.astype(f32)),
                "xq": xT[("q", g)],
                "xk": xT[("k", g)],
                "xv": xT[("v", g)],
            }
        )
    return in_maps


def run(inputs, trace=False):
    from concourse.bass_utils import run_bass_kernel_spmd

    if "nc" not in _CACHE:
        _CACHE["nc"] = _build_nc()
    nc = _CACHE["nc"]
    in_maps = _prep_inputs(
        inputs["query"], inputs["key_"], inputs["value"],
        inputs["w_q"], inputs["b_q"], inputs["w_k"], inputs["b_k"],
        inputs["w_v"], inputs["b_v"], inputs["w_o"], inputs["b_o"],
    )
    res = run_bass_kernel_spmd(
        nc, in_maps, core_ids=list(range(N_CORES)), trace=trace,
    )
    bo = np.asarray(inputs["b_o"], np.float32)
    out = np.empty((B, S, D), np.float32)
    for c in range(N_CORES):
        g, p = c // 4, c % 4
        # RS half i scatters q rows [1024*i + 256*p, 1024*i + 256*(p+1))
        core_out = np.asarray(res.results[c]["out"], np.float32)
        out[g, 256 * p : 256 * (p + 1), :] = core_out[0:256] + bo
        out[g, 1024 + 256 * p : 1024 + 256 * (p + 1), :] = core_out[256:512] + bo
    return out, res


def kernel(**inputs):
    out, _ = run(inputs, trace=False)
    return out


# revision 5
# speedup vs baseline: 1.0006x; 1.0006x over previous
"""Causal multi-head attention on 8 Trainium2 NeuronCores — v2.

Sharding: core c -> (batch g = c // 4, head-group p = c % 4, heads 4p..4p+3).
All matmuls bf16 (f32 PSUM accumulation). Causal work tiled at 128-row
k-tile granularity (fully-masked tiles skipped). Per k-tile the scores land
transposed [kpos, q] in PSUM, exp runs on the scalar engine into bf16
probs, and AV accumulates ctx^T[dk+1, q] per head (one PSUM bank per head:
a bank must never hold two open accumulation groups) with a ones column on
V putting the softmax denominators in partition 64; normalization is a DVE
reciprocal + gpsimd partition_broadcast + DVE multiply into the oproj
stationary layout. V is projected into natural [kpos, feat] layout from
resident xv. Output-projection partials are copied to bf16 SBUF, DMA'd to
DRAM per 512-row wave, ReduceScattered over each 4-core batch group, and
b_o is added on host. A budgeted interleaver paces the scores stream to
the activation engine's exp rate and fills the in-order PE with V/AV/oproj
units so no engine stalls the others.
"""

import os as os_mod
import numpy as np

B, S, D, H = 2, 2048, 1024, 16
DK = D // H
N_CORES = 8
FPC = 256  # features (head dims) per core

_CACHE = {}


def _build_nc():
    import concourse.mybir as mybir
    import concourse.tile as tile
    from concourse import bacc

    F32 = mybir.dt.float32
    BF16 = mybir.dt.bfloat16
    Exp = mybir.ActivationFunctionType.Exp

    nc = bacc.Bacc("TRN2", target_bir_lowering=False, debug=False, num_devices=8)

    cA = nc.dram_tensor("cA", [128, 2048], BF16, kind="ExternalInput")  # wq
    cB1 = nc.dram_tensor("cB1", [128, 2048], BF16, kind="ExternalInput")  # wk
    cB2 = nc.dram_tensor("cB2", [128, 4736], BF16, kind="ExternalInput")  # wv|wo|mask4|ident
    cF = nc.dram_tensor("cF", [128, 260], F32, kind="ExternalInput")  # bq|bk|bvt
    xq = nc.dram_tensor("xq", [D, S], BF16, kind="ExternalInput")
    xk = nc.dram_tensor("xk", [D, S], BF16, kind="ExternalInput")
    xv = nc.dram_tensor("xv", [D, S], BF16, kind="ExternalInput")
    out = nc.dram_tensor("out", [512, D], BF16, kind="ExternalOutput")

    no_rs = bool(os_mod.environ.get("BASS_SIM_NO_RS"))

    with tile.TileContext(nc) as tc:
        with (
            tc.tile_pool(name="consts", bufs=1) as consts,
            tc.tile_pool(name="persist", bufs=1) as persist,
            tc.tile_pool(name="xin", bufs=6) as xin,
            tc.tile_pool(name="prp", bufs=1) as prp,
            tc.tile_pool(name="small", bufs=3) as small,
            tc.tile_pool(name="oout", bufs=6) as oout,
            tc.tile_pool(name="dram", bufs=1, space="DRAM") as dram,
        ):
            # ---------------- constants ----------------
            cA_s = consts.tile([128, 2048], BF16, tag="cA", name="cA_s")
            cB1_s = consts.tile([128, 2048], BF16, tag="cB1", name="cB1_s")
            cB2_s = consts.tile([128, 4736], BF16, tag="cB2", name="cB2_s")
            cF_s = consts.tile([128, 260], F32, tag="cF", name="cF_s")
            wq_s = cA_s[:].rearrange("p (kc f) -> p kc f", kc=8)
            wk_s = cB1_s[:].rearrange("p (kc f) -> p kc f", kc=8)
            wv_s = cB2_s[:, 0:2048].rearrange("p (kc f) -> p kc f", kc=8)
            wo_s = cB2_s[:, 2048:4096].rearrange("p (c d) -> p c d", c=2)
            mask4_s = cB2_s[:, 4096:4608].rearrange("p (h x) -> p h x", h=4)
            ident_s = cB2_s[:, 4608:4736]
            bq_s = cF_s[:, 0:2]
            bk_s = cF_s[:, 2:4]
            bvt_s = cF_s[:, 4:260].rearrange("p (h x) -> p h x", h=4)

            # ---------------- persistent activations ----------------
            qT_s = persist.tile([128, 2, S], BF16, tag="qT", name="qT_s")
            kT_s = persist.tile([128, 2, S], BF16, tag="kT", name="kT_s")
            xv_s = persist.tile([128, 8, S], BF16, tag="xv", name="xv_s")
            v_s = persist.tile([128, 16, 4, 65], BF16, tag="v", name="v_s")
            ctxT_s = persist.tile([128, 2, S], BF16, tag="ctxT", name="ctxT_s")

            rs_in = [dram.tile([S // 2, D], BF16, name=f"rs_in{i}") for i in range(2)]
            rs_out = [dram.tile([256, D], BF16, name=f"rs_out{i}") for i in range(2)]

            # ones columns for the softmax denominators
            for h in range(4):
                nc.gpsimd.memset(v_s[:, :, h, 64:65], 1.0)

            # ---------------- input DMA stream (SP queue order) ----------------
            xq_t, xk_t = [], []
            nc.sync.dma_start(cA_s[:, 0:256], cA[:, 0:256])
            t = xin.tile([128, S], BF16, tag="x", name="xq0")
            nc.sync.dma_start(t[:], xq[0:128, :])
            xq_t.append(t)
            nc.sync.dma_start(cA_s[:, 256:2048], cA[:, 256:2048])
            for kc in range(1, 8):
                t = xin.tile([128, S], BF16, tag="x", name=f"xq{kc}")
                nc.sync.dma_start(t[:], xq[128 * kc : 128 * (kc + 1), :])
                xq_t.append(t)
            nc.sync.dma_start(cB1_s[:], cB1.ap())
            nc.sync.dma_start(cF_s[:], cF.ap())
            for kc in range(8):
                t = xin.tile([128, S], BF16, tag="x", name=f"xk{kc}")
                nc.sync.dma_start(t[:], xk[128 * kc : 128 * (kc + 1), :])
                xk_t.append(t)
            nc.sync.dma_start(cB2_s[:], cB2.ap())
            for kc in range(8):
                nc.sync.dma_start(xv_s[:, kc, :], xv[128 * kc : 128 * (kc + 1), :])

            # ---------------- phase 1: Q/K projections ----------------
            def proj_pass(x_t, w_s, b_s, outT, psP, split_adds=False):
                ps = {}
                for pt in range(2):
                    for qb in range(4):
                        ps[(pt, qb)] = psP.tile(
                            [128, 512], F32, tag="pp", name=f"ps{pt}{qb}"
                        )
                for kc in range(7):
                    for pt in range(2):
                        for qb in range(4):
                            nc.tensor.matmul(
                                ps[(pt, qb)][:],
                                w_s[:, kc, 128 * pt : 128 * (pt + 1)],
                                x_t[kc][:, 512 * qb : 512 * (qb + 1)],
                                start=(kc == 0),
                                stop=False,
                            )
                # final contraction step: emit the bias-add right after each
                # accumulator stops so the adds pipeline with the sweep
                for i, (pt, qb) in enumerate(
                    [(pt, qb) for pt in range(2) for qb in range(4)]
                ):
                    nc.tensor.matmul(
                        ps[(pt, qb)][:],
                        w_s[:, 7, 128 * pt : 128 * (pt + 1)],
                        x_t[7][:, 512 * qb : 512 * (qb + 1)],
                        start=False,
                        stop=True,
                    )
                    nc.vector.tensor_scalar_add(
                        outT[:, pt, 512 * qb : 512 * (qb + 1)],
                        ps[(pt, qb)][:],
                        b_s[:, pt : pt + 1],
                    )

            with tc.tile_pool(name="psP", bufs=8, space="PSUM") as psP:
                proj_pass(xq_t, wq_s, bq_s, qT_s, psP)
                proj_pass(xk_t, wk_s, bk_s, kT_s, psP, split_adds=True)

            # ---------------- phase 2: attention ----------------
            pr_t = {}  # (w, ki) -> probs tile [128, 4, 512] bf16 (wave q coords)
            prm_t = {}  # (w, diag ki) -> masked probs [128, 4, 128] bf16
            ctx_t = {}  # (w, qt) -> ctx psum tile [128, 4, 128] f32
            ctxn_t = {}  # (w, qt) -> normalized ctx sbuf [128, 4, 64] bf16

            psS_pool = [None]
            psC_pool = [None]

            def sc_unit(w, ki):
                """scores + exp (+ diag mask) for (wave, ktile), one
                head-pair (2-bank) PSUM tile per exp so PE rarely waits."""
                qoff = 128 * max(ki - 4 * w, 0)
                wdt = 512 - qoff
                pr = prp.tile(
                    [128, 4, 512], BF16, tag="pr", bufs=19, name=f"pr_{w}_{ki}"
                )
                pr_t[(w, ki)] = pr
                for hp in range(2):
                    sc = psS_pool[0].tile([128, 2, 512], F32, tag="sc", name="sc")
                    for j in range(2):
                        h = 2 * hp + j
                        r, pt = 64 * (h % 2), h // 2
                        nc.tensor.matmul(
                            sc[:, j, 0:wdt],
                            kT_s[r : r + 64, pt, 128 * ki : 128 * (ki + 1)],
                            qT_s[r : r + 64, pt, 512 * w + qoff : 512 * (w + 1)],
                            start=True,
                            stop=True,
                        )
                    nc.scalar.activation(
                        out=pr[:, 2 * hp : 2 * hp + 2, qoff:512],
                        in_=sc[:, :, 0:wdt],
                        func=Exp,
                        scale=0.125,
                    )
                if ki >= 4 * w:  # diag ktile: mask the upper triangle in place
                    nc.vector.tensor_mul(
                        pr[:, :, qoff : qoff + 128],
                        pr[:, :, qoff : qoff + 128],
                        mask4_s,
                    )

            def v_unit(st):
                pv = psV_pool[0].tile([128, 256], F32, tag="pv", name="pv")
                for kc in range(8):
                    nc.tensor.matmul(
                        pv[:],
                        xv_s[:, kc, 128 * st : 128 * (st + 1)],
                        wv_s[:, kc, :],
                        start=(kc == 0),
                        stop=(kc == 7),
                    )
                nc.vector.tensor_add(
                    v_s[:, st, :, 0:64],
                    pv[:].rearrange("p (h x) -> p h x", x=64),
                    bvt_s,
                )

            def av_open(w):
                # one [65, 512] strip per PSUM bank: a bank must never hold
                # more than one open accumulation group (start=True on one
                # strip invalidates other strips' pending sums in the bank)
                for h in range(4):
                    ctx_t[(w, h)] = psC_pool[0].tile(
                        [65, 512], F32, tag="ctx", name=f"ctx{w}{h}"
                    )

            def av_mm(w, ki):
                """ctx^T[dk+1, q] += V_aug_h^T @ probs per head: the ones
                column of V_aug puts the softmax denominators in partition
                64."""
                qoff = 128 * max(ki - 4 * w, 0)
                last = 4 * w + 3
                for h in range(4):
                    nc.tensor.matmul(
                        ctx_t[(w, h)][:, qoff:512],
                        v_s[:, ki, h, :],
                        pr_t[(w, ki)][:, h, qoff:512],
                        start=(ki == 0),
                        stop=(ki == last),
                        skip_group_check=True,
                    )

            def av_fin(w, h):
                """normalize ctx^T by the denominators in partition 64."""
                r, pt = 64 * (h % 2), h // 2
                ctx = ctx_t[(w, h)]
                recip = small.tile([1, 512], F32, tag="recip", name="recip")
                nc.vector.reciprocal(recip[:], ctx[64:65, :])
                rbc = small.tile([64, 512], F32, tag="rbc", name="rbc")
                nc.gpsimd.partition_broadcast(rbc[:], recip[:])
                nc.vector.tensor_mul(
                    ctxT_s[r : r + 64, pt, 512 * w : 512 * (w + 1)],
                    ctx[0:64, :],
                    rbc[:],
                )

            def po_unit(w, u):
                qt, nb = u // 2, u % 2
                st = 4 * w + qt
                half = w // 2
                po = psC_pool[0].tile([128, 512], F32, tag="ctx", name="po")
                for fc in range(2):
                    nc.tensor.matmul(
                        po[:],
                        ctxT_s[:, fc, 128 * st : 128 * (st + 1)],
                        wo_s[:, fc, 512 * nb : 512 * (nb + 1)],
                        start=(fc == 0),
                        stop=(fc == 1),
                    )
                ot = oout.tile([128, 512], BF16, tag="ot", name="ot")
                if w == 3:  # tail wave: Act is done with exps by then
                    nc.scalar.activation(
                        out=ot[:], in_=po[:],
                        func=mybir.ActivationFunctionType.Copy,
                    )
                else:
                    nc.vector.tensor_copy(ot[:], po[:])
                nc.sync.dma_start(
                    rs_in[half][
                        512 * (w % 2) + 128 * qt : 512 * (w % 2) + 128 * (qt + 1),
                        512 * nb : 512 * (nb + 1),
                    ],
                    ot[:],
                )

            def rs_sim_out(half):
                # sim-mode stand-in for the RS result copy; reads only the
                # first wave's first two qtiles of rs_in[half]
                if no_rs:
                    nc.sync.dma_start(
                        out[256 * half : 256 * (half + 1), :], rs_in[half][0:256, :]
                    )

            def rs_unit(half):
                if not no_rs:
                    import concourse.mybir as mybir_mod

                    nc.gpsimd.collective_compute(
                        "ReduceScatter",
                        mybir_mod.AluOpType.add,
                        replica_groups=[[0, 1, 2, 3], [4, 5, 6, 7]],
                        ins=[rs_in[half].opt()],
                        outs=[rs_out[half].opt()],
                    )
                    nc.sync.dma_start(
                        out[256 * half : 256 * (half + 1), :], rs_out[half][:]
                    )

            # ---- budgeted interleave: scores paced by Act; filler units
            # (V proj, AV, oproj) sized to keep the PE exactly as busy as
            # Act's per-ktile pace, in dependency (= PSUM rotation) order.
            WAVE_ORDER = (0, 1, 2, 3)
            sc_list = [(w, ki) for w in WAVE_ORDER for ki in range(4 * w + 4)]
            sc_idx = {u: i for i, u in enumerate(sc_list)}

            def wdt_of(w, ki):
                return 512 - 128 * max(ki - 4 * w, 0)

            def act_cost(w, ki):  # us of Act work per sc unit
                return (4 * wdt_of(w, ki) * 0.833 + 2 * 185) / 1000.0

            fillers = []  # (pe_cost_us, gate_sc_index, fn)

            def F(cost, gate, fn, *a):
                fillers.append((cost, gate, lambda a=a: fn(*a)))

            GV = 7  # V units gated until a few sc units in (xv lands late)
            for st in range(16):
                F(0.85, GV, v_unit, st)
            for w in WAVE_ORDER:
                F(0.0, None, av_open, w)
                for ki in range(4 * w + 4):
                    F(
                        4 * wdt_of(w, ki) * 0.4167 / 1000.0,
                        min(sc_idx[(w, ki)] + 3, len(sc_list)),
                        av_mm, w, ki,
                    )
                for h in range(4):
                    F(0.0, None, av_fin, w, h)
                for u in range(8):
                    F(0.43, None, po_unit, w, u)
                    if w in (1, 3) and u == 3:
                        F(0.0, None, rs_sim_out, w // 2)
                if w in (1, 3):
                    F(0.0, None, rs_unit, w // 2)

            state = {"budget": 0.0, "sci": 0, "popped": 0}

            def pump(force=False, max_pop=None):
                while fillers:
                    if max_pop is not None and state["popped"] >= max_pop:
                        break
                    cost, gate, fn = fillers[0]
                    if gate is not None and state["sci"] < gate:
                        break
                    if not force and state["budget"] < cost:
                        break
                    fillers.pop(0)
                    fn()
                    state["popped"] += 1
                    state["budget"] -= cost

            with tc.tile_pool(name="psS", bufs=2, space="PSUM") as psS:
                psS_pool[0] = psS
                with tc.tile_pool(name="psV", bufs=4, space="PSUM") as psV:
                    psV_pool = [psV]
                    # phase A: emit scores, pumping only the 16 V-proj fillers
                    while state["popped"] < 16:
                        w, ki = sc_list[state["sci"]]
                        sc_unit(w, ki)
                        state["sci"] += 1
                        state["budget"] += 1.10 * act_cost(w, ki) - 0.85
                        pump(max_pop=16)
                with tc.tile_pool(name="psC", bufs=4, space="PSUM") as psC:
                    psC_pool[0] = psC
                    while state["sci"] < len(sc_list):
                        w, ki = sc_list[state["sci"]]
                        sc_unit(w, ki)
                        state["sci"] += 1
                        state["budget"] += 1.10 * act_cost(w, ki) - 0.85
                        pump()
                    pump(force=True)

    nc.compile()
    return nc


def _prep_inputs(query, key_, value, w_q, b_q, w_k, b_k, w_v, b_v, w_o, b_o):
    """Build the 8 per-core input maps (host-side sharding / re-layout)."""
    import ml_dtypes

    f32 = np.float32
    bf16 = ml_dtypes.bfloat16

    def pack_w(wT_slice):  # [1024, 256] -> [128, 8, 256] -> [128, 2048]
        return (
            np.ascontiguousarray(
                wT_slice.reshape(8, 128, FPC).transpose(1, 0, 2).reshape(128, 2048)
            ).astype(bf16)
        )

    r = np.arange(128)
    mask = (r[None, :] >= r[:, None]).astype(f32)  # [kpos, q] allowed: q >= k
    mask4 = np.tile(mask, (1, 4)).astype(bf16)  # [128, 512]
    ident = np.eye(128, dtype=f32).astype(bf16)

    wqT = np.asarray(w_q, f32).T
    wkT = np.asarray(w_k, f32).T
    wvT = np.asarray(w_v, f32).T
    woT = np.asarray(w_o, f32).T

    xT = {}
    for g in range(B):
        xT[("q", g)] = np.ascontiguousarray(np.asarray(query[g], f32).T).astype(bf16)
        xT[("k", g)] = np.ascontiguousarray(np.asarray(key_[g], f32).T).astype(bf16)
        xT[("v", g)] = np.ascontiguousarray(np.asarray(value[g], f32).T).astype(bf16)

    in_maps = []
    for c in range(N_CORES):
        g, p = c // 4, c % 4
        fsel = slice(FPC * p, FPC * (p + 1))
        woc = (
            np.ascontiguousarray(
                woT[fsel, :].reshape(2, 128, D).transpose(1, 0, 2).reshape(128, 2048)
            ).astype(bf16)
        )
        cB2_arr = np.concatenate(
            [pack_w(wvT[:, fsel]), woc, mask4, ident], axis=1
        )
        bq_c = np.asarray(b_q, f32)[fsel].reshape(2, 128).T
        bk_c = np.asarray(b_k, f32)[fsel].reshape(2, 128).T
        bvt = np.broadcast_to(np.asarray(b_v, f32)[fsel], (128, FPC))
        cF_arr = np.concatenate([bq_c, bk_c, bvt], axis=1)
        in_maps.append(
            {
                "cA": pack_w(wqT[:, fsel]),
                "cB1": pack_w(wkT[:, fsel]),
                "cB2": np.ascontiguousarray(cB2_arr),
                "cF": np.ascontiguousarray(cF_arr.astype(f32)),
                "xq": xT[("q", g)],
                "xk": xT[("k", g)],
                "xv": xT[("v", g)],
            }
        )
    return in_maps


def run(inputs, trace=False):
    from concourse.bass_utils import run_bass_kernel_spmd

    if "nc" not in _CACHE:
        _CACHE["nc"] = _build_nc()
    nc = _CACHE["nc"]
    in_maps = _prep_inputs(
        inputs["query"], inputs["key_"], inputs["value"],
        inputs["w_q"], inputs["b_q"], inputs["w_k"], inputs["b_k"],
        inputs["w_v"], inputs["b_v"], inputs["w_o"], inputs["b_o"],
    )
    res = run_bass_kernel_spmd(
        nc, in_maps, core_ids=list(range(N_CORES)), trace=trace,
    )
    bo = np.asarray(inputs["b_o"], np.float32)
    out = np.empty((B, S, D), np.float32)
    for c in range(N_CORES):
        g, p = c // 4, c % 4
        # RS half i scatters q rows [1024*i + 256*p, 1024*i + 256*(p+1))
        core_out = np.asarray(res.results[c]["out"], np.float32)
        out[g, 256 * p : 256 * (p + 1), :] = core_out[0:256] + bo
        out[g, 1024 + 256 * p : 1024 + 256 * (p + 1), :] = core_out[256:512] + bo
    return out, res


def kernel(**inputs):
    out, _ = run(inputs, trace=False)
    return out


# revision 6
# speedup vs baseline: 1.0020x; 1.0014x over previous
"""Causal multi-head attention on 8 Trainium2 NeuronCores — v2.

Sharding: core c -> (batch g = c // 4, head-group p = c % 4, heads 4p..4p+3).
All matmuls bf16 (f32 PSUM accumulation). Causal work tiled at 128-row
k-tile granularity (fully-masked tiles skipped). Per k-tile the scores land
transposed [kpos, q] in PSUM, exp runs on the scalar engine into bf16
probs, and AV accumulates ctx^T[dk+1, q] per head (one PSUM bank per head:
a bank must never hold two open accumulation groups) with a ones column on
V putting the softmax denominators in partition 64; normalization is a DVE
reciprocal + gpsimd partition_broadcast + DVE multiply into the oproj
stationary layout. V is projected into natural [kpos, feat] layout from
resident xv. Output-projection partials are copied to bf16 SBUF, DMA'd to
DRAM per 512-row wave, ReduceScattered over each 4-core batch group, and
b_o is added on host. A budgeted interleaver paces the scores stream to
the activation engine's exp rate and fills the in-order PE with V/AV/oproj
units so no engine stalls the others.
"""

import os as os_mod
import numpy as np

B, S, D, H = 2, 2048, 1024, 16
DK = D // H
N_CORES = 8
FPC = 256  # features (head dims) per core

_CACHE = {}


def _build_nc():
    import concourse.mybir as mybir
    import concourse.tile as tile
    from concourse import bacc

    F32 = mybir.dt.float32
    BF16 = mybir.dt.bfloat16
    Exp = mybir.ActivationFunctionType.Exp

    nc = bacc.Bacc("TRN2", target_bir_lowering=False, debug=False, num_devices=8)

    cA = nc.dram_tensor("cA", [128, 2048], BF16, kind="ExternalInput")  # wq
    cB1 = nc.dram_tensor("cB1", [128, 2048], BF16, kind="ExternalInput")  # wk
    cB2 = nc.dram_tensor("cB2", [128, 4736], BF16, kind="ExternalInput")  # wv|wo|mask4|ident
    cF = nc.dram_tensor("cF", [128, 260], F32, kind="ExternalInput")  # bq|bk|bvt
    xq = nc.dram_tensor("xq", [D, S], BF16, kind="ExternalInput")
    xk = nc.dram_tensor("xk", [D, S], BF16, kind="ExternalInput")
    xv = nc.dram_tensor("xv", [D, S], BF16, kind="ExternalInput")
    out = nc.dram_tensor("out", [512, D], BF16, kind="ExternalOutput")

    no_rs = bool(os_mod.environ.get("BASS_SIM_NO_RS"))

    with tile.TileContext(nc) as tc:
        with (
            tc.tile_pool(name="consts", bufs=1) as consts,
            tc.tile_pool(name="persist", bufs=1) as persist,
            tc.tile_pool(name="xin", bufs=6) as xin,
            tc.tile_pool(name="prp", bufs=1) as prp,
            tc.tile_pool(name="small", bufs=3) as small,
            tc.tile_pool(name="oout", bufs=6) as oout,
            tc.tile_pool(name="dram", bufs=1, space="DRAM") as dram,
        ):
            # ---------------- constants ----------------
            cA_s = consts.tile([128, 2048], BF16, tag="cA", name="cA_s")
            cB1_s = consts.tile([128, 2048], BF16, tag="cB1", name="cB1_s")
            cB2_s = consts.tile([128, 4736], BF16, tag="cB2", name="cB2_s")
            cF_s = consts.tile([128, 260], F32, tag="cF", name="cF_s")
            wq_s = cA_s[:].rearrange("p (kc f) -> p kc f", kc=8)
            wk_s = cB1_s[:].rearrange("p (kc f) -> p kc f", kc=8)
            wv_s = cB2_s[:, 0:2048].rearrange("p (kc f) -> p kc f", kc=8)
            wo_s = cB2_s[:, 2048:4096].rearrange("p (c d) -> p c d", c=2)
            mask4_s = cB2_s[:, 4096:4608].rearrange("p (h x) -> p h x", h=4)
            ident_s = cB2_s[:, 4608:4736]
            bq_s = cF_s[:, 0:2]
            bk_s = cF_s[:, 2:4]
            bvt_s = cF_s[:, 4:260].rearrange("p (h x) -> p h x", h=4)

            # ---------------- persistent activations ----------------
            qT_s = persist.tile([128, 2, S], BF16, tag="qT", name="qT_s")
            kT_s = persist.tile([128, 2, S], BF16, tag="kT", name="kT_s")
            xv_s = persist.tile([128, 8, S], BF16, tag="xv", name="xv_s")
            v_s = persist.tile([128, 16, 4, 65], BF16, tag="v", name="v_s")
            ctxT_s = persist.tile([128, 2, S], BF16, tag="ctxT", name="ctxT_s")

            rs_in = [dram.tile([S // 2, D], BF16, name=f"rs_in{i}") for i in range(2)]
            rs_out = [dram.tile([256, D], BF16, name=f"rs_out{i}") for i in range(2)]

            # ones columns for the softmax denominators
            for h in range(4):
                nc.gpsimd.memset(v_s[:, :, h, 64:65], 1.0)

            # ---------------- input DMA stream (SP queue order) ----------------
            xq_t, xk_t = [], []
            nc.sync.dma_start(cA_s[:, 0:256], cA[:, 0:256])
            t = xin.tile([128, S], BF16, tag="x", name="xq0")
            nc.sync.dma_start(t[:], xq[0:128, :])
            xq_t.append(t)
            nc.sync.dma_start(cA_s[:, 256:2048], cA[:, 256:2048])
            for kc in range(1, 8):
                t = xin.tile([128, S], BF16, tag="x", name=f"xq{kc}")
                nc.sync.dma_start(t[:], xq[128 * kc : 128 * (kc + 1), :])
                xq_t.append(t)
            nc.sync.dma_start(cB1_s[:], cB1.ap())
            nc.sync.dma_start(cF_s[:], cF.ap())
            for kc in range(8):
                t = xin.tile([128, S], BF16, tag="x", name=f"xk{kc}")
                nc.sync.dma_start(t[:], xk[128 * kc : 128 * (kc + 1), :])
                xk_t.append(t)
            nc.sync.dma_start(cB2_s[:], cB2.ap())
            for kc in range(8):
                nc.sync.dma_start(xv_s[:, kc, :], xv[128 * kc : 128 * (kc + 1), :])

            # ---------------- phase 1: Q/K projections ----------------
            def proj_pass(x_t, w_s, b_s, outT, psP, split_adds=False):
                ps = {}
                for pt in range(2):
                    for qb in range(4):
                        ps[(pt, qb)] = psP.tile(
                            [128, 512], F32, tag="pp", name=f"ps{pt}{qb}"
                        )
                for kc in range(7):
                    for pt in range(2):
                        for qb in range(4):
                            nc.tensor.matmul(
                                ps[(pt, qb)][:],
                                w_s[:, kc, 128 * pt : 128 * (pt + 1)],
                                x_t[kc][:, 512 * qb : 512 * (qb + 1)],
                                start=(kc == 0),
                                stop=False,
                            )
                # final contraction step: emit the bias-add right after each
                # accumulator stops so the adds pipeline with the sweep
                for i, (pt, qb) in enumerate(
                    [(pt, qb) for pt in range(2) for qb in range(4)]
                ):
                    nc.tensor.matmul(
                        ps[(pt, qb)][:],
                        w_s[:, 7, 128 * pt : 128 * (pt + 1)],
                        x_t[7][:, 512 * qb : 512 * (qb + 1)],
                        start=False,
                        stop=True,
                    )
                    nc.vector.tensor_scalar_add(
                        outT[:, pt, 512 * qb : 512 * (qb + 1)],
                        ps[(pt, qb)][:],
                        b_s[:, pt : pt + 1],
                    )

            with tc.tile_pool(name="psP", bufs=8, space="PSUM") as psP:
                proj_pass(xq_t, wq_s, bq_s, qT_s, psP)
                proj_pass(xk_t, wk_s, bk_s, kT_s, psP, split_adds=True)

            # ---------------- phase 2: attention ----------------
            pr_t = {}  # (w, ki) -> probs tile [128, 4, 512] bf16 (wave q coords)
            prm_t = {}  # (w, diag ki) -> masked probs [128, 4, 128] bf16
            ctx_t = {}  # (w, qt) -> ctx psum tile [128, 4, 128] f32
            ctxn_t = {}  # (w, qt) -> normalized ctx sbuf [128, 4, 64] bf16

            psS_pool = [None]
            psC_pool = [None]

            def sc_unit(w, ki):
                """scores + exp (+ diag mask) for (wave, ktile), one
                head-pair (2-bank) PSUM tile per exp so PE rarely waits."""
                qoff = 128 * max(ki - 4 * w, 0)
                wdt = 512 - qoff
                pr = prp.tile(
                    [128, 4, 512], BF16, tag="pr", bufs=19, name=f"pr_{w}_{ki}"
                )
                pr_t[(w, ki)] = pr
                for hp in range(2):
                    sc = psS_pool[0].tile([128, 2, 512], F32, tag="sc", name="sc")
                    for j in range(2):
                        h = 2 * hp + j
                        r, pt = 64 * (h % 2), h // 2
                        nc.tensor.matmul(
                            sc[:, j, 0:wdt],
                            kT_s[r : r + 64, pt, 128 * ki : 128 * (ki + 1)],
                            qT_s[r : r + 64, pt, 512 * w + qoff : 512 * (w + 1)],
                            start=True,
                            stop=True,
                        )
                    nc.scalar.activation(
                        out=pr[:, 2 * hp : 2 * hp + 2, qoff:512],
                        in_=sc[:, :, 0:wdt],
                        func=Exp,
                        scale=0.125,
                    )
                if ki >= 4 * w:  # diag ktile: mask the upper triangle in place
                    nc.vector.tensor_mul(
                        pr[:, :, qoff : qoff + 128],
                        pr[:, :, qoff : qoff + 128],
                        mask4_s,
                    )

            def v_unit(st):
                pv = psV_pool[0].tile([128, 256], F32, tag="pv", name="pv")
                for kc in range(8):
                    nc.tensor.matmul(
                        pv[:],
                        xv_s[:, kc, 128 * st : 128 * (st + 1)],
                        wv_s[:, kc, :],
                        start=(kc == 0),
                        stop=(kc == 7),
                    )
                nc.vector.tensor_add(
                    v_s[:, st, :, 0:64],
                    pv[:].rearrange("p (h x) -> p h x", x=64),
                    bvt_s,
                )

            def av_open(w):
                # one [65, 512] strip per PSUM bank: a bank must never hold
                # more than one open accumulation group (start=True on one
                # strip invalidates other strips' pending sums in the bank)
                for h in range(4):
                    ctx_t[(w, h)] = psC_pool[0].tile(
                        [65, 512], F32, tag="ctx", name=f"ctx{w}{h}"
                    )

            def av_mm(w, ki):
                """ctx^T[dk+1, q] += V_aug_h^T @ probs per head: the ones
                column of V_aug puts the softmax denominators in partition
                64."""
                qoff = 128 * max(ki - 4 * w, 0)
                last = 4 * w + 3
                for h in range(4):
                    nc.tensor.matmul(
                        ctx_t[(w, h)][:, qoff:512],
                        v_s[:, ki, h, :],
                        pr_t[(w, ki)][:, h, qoff:512],
                        start=(ki == 0),
                        stop=(ki == last),
                        skip_group_check=True,
                    )

            def av_fin(w, h):
                """normalize ctx^T by the denominators in partition 64."""
                r, pt = 64 * (h % 2), h // 2
                ctx = ctx_t[(w, h)]
                recip = small.tile([1, 512], F32, tag="recip", name="recip")
                nc.vector.reciprocal(recip[:], ctx[64:65, :])
                rbc = small.tile([64, 512], F32, tag="rbc", name="rbc")
                nc.gpsimd.partition_broadcast(rbc[:], recip[:])
                nc.vector.tensor_mul(
                    ctxT_s[r : r + 64, pt, 512 * w : 512 * (w + 1)],
                    ctx[0:64, :],
                    rbc[:],
                )

            def po_unit(w, u):
                qt, nb = u // 2, u % 2
                st = 4 * w + qt
                half = w // 2
                po = psC_pool[0].tile([128, 512], F32, tag="ctx", name="po")
                for fc in range(2):
                    nc.tensor.matmul(
                        po[:],
                        ctxT_s[:, fc, 128 * st : 128 * (st + 1)],
                        wo_s[:, fc, 512 * nb : 512 * (nb + 1)],
                        start=(fc == 0),
                        stop=(fc == 1),
                    )
                ot = oout.tile([128, 512], BF16, tag="ot", name="ot")
                if w == 3:  # tail wave: Act is done with exps by then
                    nc.scalar.activation(
                        out=ot[:], in_=po[:],
                        func=mybir.ActivationFunctionType.Copy,
                    )
                else:
                    nc.vector.tensor_copy(ot[:], po[:])
                nc.sync.dma_start(
                    rs_in[half][
                        512 * (w % 2) + 128 * qt : 512 * (w % 2) + 128 * (qt + 1),
                        512 * nb : 512 * (nb + 1),
                    ],
                    ot[:],
                )

            def rs_sim_out(half):
                # sim-mode stand-in for the RS result copy; reads only the
                # first wave's first two qtiles of rs_in[half]
                if no_rs:
                    nc.sync.dma_start(
                        out[256 * half : 256 * (half + 1), :], rs_in[half][0:256, :]
                    )

            def rs_unit(half):
                if not no_rs:
                    import concourse.mybir as mybir_mod

                    nc.gpsimd.collective_compute(
                        "ReduceScatter",
                        mybir_mod.AluOpType.add,
                        replica_groups=[[0, 1, 2, 3], [4, 5, 6, 7]],
                        ins=[rs_in[half].opt()],
                        outs=[rs_out[half].opt()],
                    )
                    nc.sync.dma_start(
                        out[256 * half : 256 * (half + 1), :], rs_out[half][:]
                    )

            # ---- budgeted interleave: scores paced by Act; filler units
            # (V proj, AV, oproj) sized to keep the PE exactly as busy as
            # Act's per-ktile pace, in dependency (= PSUM rotation) order.
            WAVE_ORDER = (0, 1, 2, 3)
            sc_list = [(w, ki) for w in WAVE_ORDER for ki in range(4 * w + 4)]
            sc_idx = {u: i for i, u in enumerate(sc_list)}

            def wdt_of(w, ki):
                return 512 - 128 * max(ki - 4 * w, 0)

            def act_cost(w, ki):  # us of Act work per sc unit
                return (4 * wdt_of(w, ki) * 0.833 + 2 * 185) / 1000.0

            fillers = []  # (pe_cost_us, gate_sc_index, fn)

            def F(cost, gate, fn, *a):
                fillers.append((cost, gate, lambda a=a: fn(*a)))

            GV = 13  # V units gated until a few sc units in (xv lands late)
            for st in range(16):
                F(0.85, GV, v_unit, st)
            for w in WAVE_ORDER:
                F(0.0, None, av_open, w)
                for ki in range(4 * w + 4):
                    F(
                        4 * wdt_of(w, ki) * 0.4167 / 1000.0,
                        min(sc_idx[(w, ki)] + 3, len(sc_list)),
                        av_mm, w, ki,
                    )
                for h in range(4):
                    F(0.0, None, av_fin, w, h)
                for u in range(8):
                    F(0.43, None, po_unit, w, u)
                    if w in (1, 3) and u == 3:
                        F(0.0, None, rs_sim_out, w // 2)
                if w in (1, 3):
                    F(0.0, None, rs_unit, w // 2)

            state = {"budget": 0.0, "sci": 0, "popped": 0}

            def pump(force=False, max_pop=None):
                while fillers:
                    if max_pop is not None and state["popped"] >= max_pop:
                        break
                    cost, gate, fn = fillers[0]
                    if gate is not None and state["sci"] < gate:
                        break
                    if not force and state["budget"] < cost:
                        break
                    fillers.pop(0)
                    fn()
                    state["popped"] += 1
                    state["budget"] -= cost

            with tc.tile_pool(name="psS", bufs=2, space="PSUM") as psS:
                psS_pool[0] = psS
                with tc.tile_pool(name="psV", bufs=4, space="PSUM") as psV:
                    psV_pool = [psV]
                    # phase A: emit scores, pumping only the 16 V-proj fillers
                    while state["popped"] < 16:
                        w, ki = sc_list[state["sci"]]
                        sc_unit(w, ki)
                        state["sci"] += 1
                        state["budget"] += 1.10 * act_cost(w, ki) - 0.85
                        pump(max_pop=16)
                with tc.tile_pool(name="psC", bufs=4, space="PSUM") as psC:
                    psC_pool[0] = psC
                    while state["sci"] < len(sc_list):
                        w, ki = sc_list[state["sci"]]
                        sc_unit(w, ki)
                        state["sci"] += 1
                        state["budget"] += 1.10 * act_cost(w, ki) - 0.85
                        pump()
                    pump(force=True)

    nc.compile()
    return nc


def _prep_inputs(query, key_, value, w_q, b_q, w_k, b_k, w_v, b_v, w_o, b_o):
    """Build the 8 per-core input maps (host-side sharding / re-layout)."""
    import ml_dtypes

    f32 = np.float32
    bf16 = ml_dtypes.bfloat16

    def pack_w(wT_slice):  # [1024, 256] -> [128, 8, 256] -> [128, 2048]
        return (
            np.ascontiguousarray(
                wT_slice.reshape(8, 128, FPC).transpose(1, 0, 2).reshape(128, 2048)
            ).astype(bf16)
        )

    r = np.arange(128)
    mask = (r[None, :] >= r[:, None]).astype(f32)  # [kpos, q] allowed: q >= k
    mask4 = np.tile(mask, (1, 4)).astype(bf16)  # [128, 512]
    ident = np.eye(128, dtype=f32).astype(bf16)

    wqT = np.asarray(w_q, f32).T
    wkT = np.asarray(w_k, f32).T
    wvT = np.asarray(w_v, f32).T
    woT = np.asarray(w_o, f32).T

    xT = {}
    for g in range(B):
        xT[("q", g)] = np.ascontiguousarray(np.asarray(query[g], f32).T).astype(bf16)
        xT[("k", g)] = np.ascontiguousarray(np.asarray(key_[g], f32).T).astype(bf16)
        xT[("v", g)] = np.ascontiguousarray(np.asarray(value[g], f32).T).astype(bf16)

    in_maps = []
    for c in range(N_CORES):
        g, p = c // 4, c % 4
        fsel = slice(FPC * p, FPC * (p + 1))
        woc = (
            np.ascontiguousarray(
                woT[fsel, :].reshape(2, 128, D).transpose(1, 0, 2).reshape(128, 2048)
            ).astype(bf16)
        )
        cB2_arr = np.concatenate(
            [pack_w(wvT[:, fsel]), woc, mask4, ident], axis=1
        )
        bq_c = np.asarray(b_q, f32)[fsel].reshape(2, 128).T
        bk_c = np.asarray(b_k, f32)[fsel].reshape(2, 128).T
        bvt = np.broadcast_to(np.asarray(b_v, f32)[fsel], (128, FPC))
        cF_arr = np.concatenate([bq_c, bk_c, bvt], axis=1)
        in_maps.append(
            {
                "cA": pack_w(wqT[:, fsel]),
                "cB1": pack_w(wkT[:, fsel]),
                "cB2": np.ascontiguousarray(cB2_arr),
                "cF": np.ascontiguousarray(cF_arr.astype(f32)),
                "xq": xT[("q", g)],
                "xk": xT[("k", g)],
                "xv": xT[("v", g)],
            }
        )
    return in_maps


def run(inputs, trace=False):
    from concourse.bass_utils import run_bass_kernel_spmd

    if "nc" not in _CACHE:
        _CACHE["nc"] = _build_nc()
    nc = _CACHE["nc"]
    in_maps = _prep_inputs(
        inputs["query"], inputs["key_"], inputs["value"],
        inputs["w_q"], inputs["b_q"], inputs["w_k"], inputs["b_k"],
        inputs["w_v"], inputs["b_v"], inputs["w_o"], inputs["b_o"],
    )
    res = run_bass_kernel_spmd(
        nc, in_maps, core_ids=list(range(N_CORES)), trace=trace,
    )
    bo = np.asarray(inputs["b_o"], np.float32)
    out = np.empty((B, S, D), np.float32)
    for c in range(N_CORES):
        g, p = c // 4, c % 4
        # RS half i scatters q rows [1024*i + 256*p, 1024*i + 256*(p+1))
        core_out = np.asarray(res.results[c]["out"], np.float32)
        out[g, 256 * p : 256 * (p + 1), :] = core_out[0:256] + bo
        out[g, 1024 + 256 * p : 1024 + 256 * (p + 1), :] = core_out[256:512] + bo
    return out, res


def kernel(**inputs):
    out, _ = run(inputs, trace=False)
    return out


# revision 7
# speedup vs baseline: 1.0111x; 1.0090x over previous
"""Causal multi-head attention on 8 Trainium2 NeuronCores — v2.

Sharding: core c -> (batch g = c // 4, head-group p = c % 4, heads 4p..4p+3).
All matmuls bf16 (f32 PSUM accumulation). Causal work tiled at 128-row
k-tile granularity (fully-masked tiles skipped). Per k-tile the scores land
transposed [kpos, q] in PSUM, exp runs on the scalar engine into bf16
probs, and AV accumulates ctx^T[dk+1, q] per head (one PSUM bank per head:
a bank must never hold two open accumulation groups) with a ones column on
V putting the softmax denominators in partition 64; normalization is a DVE
reciprocal + gpsimd partition_broadcast + DVE multiply into the oproj
stationary layout. V is projected into natural [kpos, feat] layout from
resident xv. Output-projection partials are copied to bf16 SBUF, DMA'd to
DRAM per 512-row wave, ReduceScattered over each 4-core batch group, and
b_o is added on host. A budgeted interleaver paces the scores stream to
the activation engine's exp rate and fills the in-order PE with V/AV/oproj
units so no engine stalls the others.
"""

import os as os_mod
import numpy as np

B, S, D, H = 2, 2048, 1024, 16
DK = D // H
N_CORES = 8
FPC = 256  # features (head dims) per core

_CACHE = {}


def _build_nc():
    import concourse.mybir as mybir
    import concourse.tile as tile
    from concourse import bacc

    F32 = mybir.dt.float32
    BF16 = mybir.dt.bfloat16
    Exp = mybir.ActivationFunctionType.Exp

    nc = bacc.Bacc("TRN2", target_bir_lowering=False, debug=False, num_devices=8)

    cA = nc.dram_tensor("cA", [128, 2048], BF16, kind="ExternalInput")  # wq
    cB1 = nc.dram_tensor("cB1", [128, 2048], BF16, kind="ExternalInput")  # wk
    cB2 = nc.dram_tensor("cB2", [128, 4736], BF16, kind="ExternalInput")  # wv|wo|mask4|ident
    cF = nc.dram_tensor("cF", [128, 260], F32, kind="ExternalInput")  # bq|bk|bvt
    xq = nc.dram_tensor("xq", [D, S], BF16, kind="ExternalInput")
    xk = nc.dram_tensor("xk", [D, S], BF16, kind="ExternalInput")
    xv = nc.dram_tensor("xv", [D, S], BF16, kind="ExternalInput")
    out = nc.dram_tensor("out", [512, D], BF16, kind="ExternalOutput")

    no_rs = bool(os_mod.environ.get("BASS_SIM_NO_RS"))

    with tile.TileContext(nc) as tc:
        with (
            tc.tile_pool(name="consts", bufs=1) as consts,
            tc.tile_pool(name="persist", bufs=1) as persist,
            tc.tile_pool(name="xin", bufs=6) as xin,
            tc.tile_pool(name="prp", bufs=1) as prp,
            tc.tile_pool(name="small", bufs=3) as small,
            tc.tile_pool(name="oout", bufs=6) as oout,
            tc.tile_pool(name="dram", bufs=1, space="DRAM") as dram,
        ):
            # ---------------- constants ----------------
            cA_s = consts.tile([128, 2048], BF16, tag="cA", name="cA_s")
            cB1_s = consts.tile([128, 2048], BF16, tag="cB1", name="cB1_s")
            cB2_s = consts.tile([128, 4736], BF16, tag="cB2", name="cB2_s")
            cF_s = consts.tile([128, 260], F32, tag="cF", name="cF_s")
            wq_s = cA_s[:].rearrange("p (kc f) -> p kc f", kc=8)
            wk_s = cB1_s[:].rearrange("p (kc f) -> p kc f", kc=8)
            wv_s = cB2_s[:, 0:2048].rearrange("p (kc f) -> p kc f", kc=8)
            wo_s = cB2_s[:, 2048:4096].rearrange("p (c d) -> p c d", c=2)
            mask4_s = cB2_s[:, 4096:4608].rearrange("p (h x) -> p h x", h=4)
            ident_s = cB2_s[:, 4608:4736]
            bq_s = cF_s[:, 0:2]
            bk_s = cF_s[:, 2:4]
            bvt_s = cF_s[:, 4:260].rearrange("p (h x) -> p h x", h=4)

            # ---------------- persistent activations ----------------
            qT_s = persist.tile([128, 2, S], BF16, tag="qT", name="qT_s")
            kT_s = persist.tile([128, 2, S], BF16, tag="kT", name="kT_s")
            xv_s = persist.tile([128, 8, S], BF16, tag="xv", name="xv_s")
            v_s = persist.tile([128, 16, 4, 65], BF16, tag="v", name="v_s")
            ctxT_s = persist.tile([128, 2, S], BF16, tag="ctxT", name="ctxT_s")

            rs_in = [dram.tile([S // 2, D], BF16, name=f"rs_in{i}") for i in range(2)]
            rs_out = [dram.tile([256, D], BF16, name=f"rs_out{i}") for i in range(2)]

            # ones columns for the softmax denominators
            for h in range(4):
                nc.gpsimd.memset(v_s[:, :, h, 64:65], 1.0)

            # warm the Exp table at t=0 so LoadActFuncSet is off the
            # first-exp critical path
            warm = small.tile([1, 8], F32, tag="warm", bufs=1, name="warm")
            nc.vector.memset(warm[:], 0.0)
            nc.scalar.activation(out=warm[:], in_=warm[:], func=Exp)

            # ---------------- input DMA stream (SP queue order) ----------------
            xq_t, xk_t = [], []
            nc.sync.dma_start(cA_s[:, 0:256], cA[:, 0:256])
            t = xin.tile([128, S], BF16, tag="x", name="xq0")
            nc.sync.dma_start(t[:], xq[0:128, :])
            xq_t.append(t)
            nc.sync.dma_start(cA_s[:, 256:2048], cA[:, 256:2048])
            for kc in range(1, 8):
                t = xin.tile([128, S], BF16, tag="x", name=f"xq{kc}")
                nc.sync.dma_start(t[:], xq[128 * kc : 128 * (kc + 1), :])
                xq_t.append(t)
            nc.sync.dma_start(cB1_s[:], cB1.ap())
            nc.sync.dma_start(cF_s[:], cF.ap())
            for kc in range(8):
                t = xin.tile([128, S], BF16, tag="x", name=f"xk{kc}")
                nc.sync.dma_start(t[:], xk[128 * kc : 128 * (kc + 1), :])
                xk_t.append(t)
            nc.sync.dma_start(cB2_s[:], cB2.ap())
            for kc in range(8):
                nc.sync.dma_start(xv_s[:, kc, :], xv[128 * kc : 128 * (kc + 1), :])

            # ---------------- phase 1: Q/K projections ----------------
            def proj_pass(x_t, w_s, b_s, outT, psP, split_adds=False):
                ps = {}
                for pt in range(2):
                    for qb in range(4):
                        ps[(pt, qb)] = psP.tile(
                            [128, 512], F32, tag="pp", name=f"ps{pt}{qb}"
                        )
                for kc in range(7):
                    for pt in range(2):
                        for qb in range(4):
                            nc.tensor.matmul(
                                ps[(pt, qb)][:],
                                w_s[:, kc, 128 * pt : 128 * (pt + 1)],
                                x_t[kc][:, 512 * qb : 512 * (qb + 1)],
                                start=(kc == 0),
                                stop=False,
                            )
                # final contraction step: emit the bias-add right after each
                # accumulator stops so the adds pipeline with the sweep
                for i, (pt, qb) in enumerate(
                    [(pt, qb) for pt in range(2) for qb in range(4)]
                ):
                    nc.tensor.matmul(
                        ps[(pt, qb)][:],
                        w_s[:, 7, 128 * pt : 128 * (pt + 1)],
                        x_t[7][:, 512 * qb : 512 * (qb + 1)],
                        start=False,
                        stop=True,
                    )
                    nc.vector.tensor_scalar_add(
                        outT[:, pt, 512 * qb : 512 * (qb + 1)],
                        ps[(pt, qb)][:],
                        b_s[:, pt : pt + 1],
                    )

            with tc.tile_pool(name="psP", bufs=8, space="PSUM") as psP:
                proj_pass(xq_t, wq_s, bq_s, qT_s, psP)
                proj_pass(xk_t, wk_s, bk_s, kT_s, psP, split_adds=True)

            # ---------------- phase 2: attention ----------------
            pr_t = {}  # (w, ki) -> probs tile [128, 4, 512] bf16 (wave q coords)
            prm_t = {}  # (w, diag ki) -> masked probs [128, 4, 128] bf16
            ctx_t = {}  # (w, qt) -> ctx psum tile [128, 4, 128] f32
            ctxn_t = {}  # (w, qt) -> normalized ctx sbuf [128, 4, 64] bf16

            psS_pool = [None]
            psC_pool = [None]

            def sc_unit(w, ki):
                """scores + exp (+ diag mask) for (wave, ktile), one
                head-pair (2-bank) PSUM tile per exp so PE rarely waits."""
                qoff = 128 * max(ki - 4 * w, 0)
                wdt = 512 - qoff
                pr = prp.tile(
                    [128, 4, 512], BF16, tag="pr", bufs=19, name=f"pr_{w}_{ki}"
                )
                pr_t[(w, ki)] = pr
                for hp in range(2):
                    sc = psS_pool[0].tile([128, 2, 512], F32, tag="sc", name="sc")
                    for j in range(2):
                        h = 2 * hp + j
                        r, pt = 64 * (h % 2), h // 2
                        nc.tensor.matmul(
                            sc[:, j, 0:wdt],
                            kT_s[r : r + 64, pt, 128 * ki : 128 * (ki + 1)],
                            qT_s[r : r + 64, pt, 512 * w + qoff : 512 * (w + 1)],
                            start=True,
                            stop=True,
                        )
                    nc.scalar.activation(
                        out=pr[:, 2 * hp : 2 * hp + 2, qoff:512],
                        in_=sc[:, :, 0:wdt],
                        func=Exp,
                        scale=0.125,
                    )
                if ki >= 4 * w:  # diag ktile: mask the upper triangle in place
                    nc.vector.tensor_mul(
                        pr[:, :, qoff : qoff + 128],
                        pr[:, :, qoff : qoff + 128],
                        mask4_s,
                    )

            def v_unit(st):
                pv = psV_pool[0].tile([128, 256], F32, tag="pv", name="pv")
                for kc in range(8):
                    nc.tensor.matmul(
                        pv[:],
                        xv_s[:, kc, 128 * st : 128 * (st + 1)],
                        wv_s[:, kc, :],
                        start=(kc == 0),
                        stop=(kc == 7),
                    )
                nc.vector.tensor_add(
                    v_s[:, st, :, 0:64],
                    pv[:].rearrange("p (h x) -> p h x", x=64),
                    bvt_s,
                )

            def av_open(w):
                # one [65, 512] strip per PSUM bank: a bank must never hold
                # more than one open accumulation group (start=True on one
                # strip invalidates other strips' pending sums in the bank)
                for h in range(4):
                    ctx_t[(w, h)] = psC_pool[0].tile(
                        [65, 512], F32, tag="ctx", name=f"ctx{w}{h}"
                    )

            def av_mm(w, ki):
                """ctx^T[dk+1, q] += V_aug_h^T @ probs per head: the ones
                column of V_aug puts the softmax denominators in partition
                64."""
                qoff = 128 * max(ki - 4 * w, 0)
                last = 4 * w + 3
                for h in range(4):
                    nc.tensor.matmul(
                        ctx_t[(w, h)][:, qoff:512],
                        v_s[:, ki, h, :],
                        pr_t[(w, ki)][:, h, qoff:512],
                        start=(ki == 0),
                        stop=(ki == last),
                        skip_group_check=True,
                    )

            def av_fin(w, h):
                """normalize ctx^T by the denominators in partition 64."""
                r, pt = 64 * (h % 2), h // 2
                ctx = ctx_t[(w, h)]
                recip = small.tile([1, 512], F32, tag="recip", name="recip")
                nc.vector.reciprocal(recip[:], ctx[64:65, :])
                rbc = small.tile([64, 512], F32, tag="rbc", name="rbc")
                nc.gpsimd.partition_broadcast(rbc[:], recip[:])
                nc.vector.tensor_mul(
                    ctxT_s[r : r + 64, pt, 512 * w : 512 * (w + 1)],
                    ctx[0:64, :],
                    rbc[:],
                )

            def po_unit(w, u):
                qt, nb = u // 2, u % 2
                st = 4 * w + qt
                half = w // 2
                po = psC_pool[0].tile([128, 512], F32, tag="ctx", name="po")
                for fc in range(2):
                    nc.tensor.matmul(
                        po[:],
                        ctxT_s[:, fc, 128 * st : 128 * (st + 1)],
                        wo_s[:, fc, 512 * nb : 512 * (nb + 1)],
                        start=(fc == 0),
                        stop=(fc == 1),
                    )
                ot = oout.tile([128, 512], BF16, tag="ot", name="ot")
                if w == 3:  # tail wave: Act is done with exps by then
                    nc.scalar.activation(
                        out=ot[:], in_=po[:],
                        func=mybir.ActivationFunctionType.Copy,
                    )
                else:
                    nc.vector.tensor_copy(ot[:], po[:])
                nc.sync.dma_start(
                    rs_in[half][
                        512 * (w % 2) + 128 * qt : 512 * (w % 2) + 128 * (qt + 1),
                        512 * nb : 512 * (nb + 1),
                    ],
                    ot[:],
                )

            def rs_sim_out(half):
                # sim-mode stand-in for the RS result copy; reads only the
                # first wave's first two qtiles of rs_in[half]
                if no_rs:
                    nc.sync.dma_start(
                        out[256 * half : 256 * (half + 1), :], rs_in[half][0:256, :]
                    )

            def rs_unit(half):
                if not no_rs:
                    import concourse.mybir as mybir_mod

                    nc.gpsimd.collective_compute(
                        "ReduceScatter",
                        mybir_mod.AluOpType.add,
                        replica_groups=[[0, 1, 2, 3], [4, 5, 6, 7]],
                        ins=[rs_in[half].opt()],
                        outs=[rs_out[half].opt()],
                    )
                    nc.sync.dma_start(
                        out[256 * half : 256 * (half + 1), :], rs_out[half][:]
                    )

            # ---- budgeted interleave: scores paced by Act; filler units
            # (V proj, AV, oproj) sized to keep the PE exactly as busy as
            # Act's per-ktile pace, in dependency (= PSUM rotation) order.
            WAVE_ORDER = (0, 1, 2, 3)
            sc_list = [(w, ki) for w in WAVE_ORDER for ki in range(4 * w + 4)]
            sc_idx = {u: i for i, u in enumerate(sc_list)}

            def wdt_of(w, ki):
                return 512 - 128 * max(ki - 4 * w, 0)

            def act_cost(w, ki):  # us of Act work per sc unit
                return (4 * wdt_of(w, ki) * 0.833 + 2 * 185) / 1000.0

            fillers = []  # (pe_cost_us, gate_sc_index, fn)

            def F(cost, gate, fn, *a):
                fillers.append((cost, gate, lambda a=a: fn(*a)))

            GV = 13  # V units gated until a few sc units in (xv lands late)
            for st in range(16):
                F(0.85, GV, v_unit, st)
            for w in WAVE_ORDER:
                F(0.0, None, av_open, w)
                for ki in range(4 * w + 4):
                    F(
                        4 * wdt_of(w, ki) * 0.4167 / 1000.0,
                        min(sc_idx[(w, ki)] + 3, len(sc_list)),
                        av_mm, w, ki,
                    )
                for h in range(4):
                    F(0.0, None, av_fin, w, h)
                for u in range(8):
                    F(0.43, None, po_unit, w, u)
                    if w in (0, 2) and u == 3:
                        F(0.0, None, rs_sim_out, w // 2)
                if w in (1, 3):
                    F(0.0, None, rs_unit, w // 2)

            state = {"budget": 0.0, "sci": 0, "popped": 0}

            def pump(force=False, max_pop=None):
                while fillers:
                    if max_pop is not None and state["popped"] >= max_pop:
                        break
                    cost, gate, fn = fillers[0]
                    if gate is not None and state["sci"] < gate:
                        break
                    if not force and state["budget"] < cost:
                        break
                    fillers.pop(0)
                    fn()
                    state["popped"] += 1
                    state["budget"] -= cost

            with tc.tile_pool(name="psS", bufs=2, space="PSUM") as psS:
                psS_pool[0] = psS
                with tc.tile_pool(name="psV", bufs=4, space="PSUM") as psV:
                    psV_pool = [psV]
                    # phase A: emit scores, pumping only the 16 V-proj fillers
                    while state["popped"] < 16:
                        w, ki = sc_list[state["sci"]]
                        sc_unit(w, ki)
                        state["sci"] += 1
                        state["budget"] += 1.10 * act_cost(w, ki) - 0.85
                        pump(max_pop=16)
                with tc.tile_pool(name="psC", bufs=4, space="PSUM") as psC:
                    psC_pool[0] = psC
                    while state["sci"] < len(sc_list):
                        w, ki = sc_list[state["sci"]]
                        sc_unit(w, ki)
                        state["sci"] += 1
                        state["budget"] += 1.10 * act_cost(w, ki) - 0.85
                        pump()
                    pump(force=True)

    nc.compile()
    return nc


def _prep_inputs(query, key_, value, w_q, b_q, w_k, b_k, w_v, b_v, w_o, b_o):
    """Build the 8 per-core input maps (host-side sharding / re-layout)."""
    import ml_dtypes

    f32 = np.float32
    bf16 = ml_dtypes.bfloat16

    def pack_w(wT_slice):  # [1024, 256] -> [128, 8, 256] -> [128, 2048]
        return (
            np.ascontiguousarray(
                wT_slice.reshape(8, 128, FPC).transpose(1, 0, 2).reshape(128, 2048)
            ).astype(bf16)
        )

    r = np.arange(128)
    mask = (r[None, :] >= r[:, None]).astype(f32)  # [kpos, q] allowed: q >= k
    mask4 = np.tile(mask, (1, 4)).astype(bf16)  # [128, 512]
    ident = np.eye(128, dtype=f32).astype(bf16)

    wqT = np.asarray(w_q, f32).T
    wkT = np.asarray(w_k, f32).T
    wvT = np.asarray(w_v, f32).T
    woT = np.asarray(w_o, f32).T

    xT = {}
    for g in range(B):
        xT[("q", g)] = np.ascontiguousarray(np.asarray(query[g], f32).T).astype(bf16)
        xT[("k", g)] = np.ascontiguousarray(np.asarray(key_[g], f32).T).astype(bf16)
        xT[("v", g)] = np.ascontiguousarray(np.asarray(value[g], f32).T).astype(bf16)

    in_maps = []
    for c in range(N_CORES):
        g, p = c // 4, c % 4
        fsel = slice(FPC * p, FPC * (p + 1))
        woc = (
            np.ascontiguousarray(
                woT[fsel, :].reshape(2, 128, D).transpose(1, 0, 2).reshape(128, 2048)
            ).astype(bf16)
        )
        cB2_arr = np.concatenate(
            [pack_w(wvT[:, fsel]), woc, mask4, ident], axis=1
        )
        bq_c = np.asarray(b_q, f32)[fsel].reshape(2, 128).T
        bk_c = np.asarray(b_k, f32)[fsel].reshape(2, 128).T
        bvt = np.broadcast_to(np.asarray(b_v, f32)[fsel], (128, FPC))
        cF_arr = np.concatenate([bq_c, bk_c, bvt], axis=1)
        in_maps.append(
            {
                "cA": pack_w(wqT[:, fsel]),
                "cB1": pack_w(wkT[:, fsel]),
                "cB2": np.ascontiguousarray(cB2_arr),
                "cF": np.ascontiguousarray(cF_arr.astype(f32)),
                "xq": xT[("q", g)],
                "xk": xT[("k", g)],
                "xv": xT[("v", g)],
            }
        )
    return in_maps


def run(inputs, trace=False):
    from concourse.bass_utils import run_bass_kernel_spmd

    if "nc" not in _CACHE:
        _CACHE["nc"] = _build_nc()
    nc = _CACHE["nc"]
    in_maps = _prep_inputs(
        inputs["query"], inputs["key_"], inputs["value"],
        inputs["w_q"], inputs["b_q"], inputs["w_k"], inputs["b_k"],
        inputs["w_v"], inputs["b_v"], inputs["w_o"], inputs["b_o"],
    )
    res = run_bass_kernel_spmd(
        nc, in_maps, core_ids=list(range(N_CORES)), trace=trace,
    )
    bo = np.asarray(inputs["b_o"], np.float32)
    out = np.empty((B, S, D), np.float32)
    for c in range(N_CORES):
        g, p = c // 4, c % 4
        # RS half i scatters q rows [1024*i + 256*p, 1024*i + 256*(p+1))
        core_out = np.asarray(res.results[c]["out"], np.float32)
        out[g, 256 * p : 256 * (p + 1), :] = core_out[0:256] + bo
        out[g, 1024 + 256 * p : 1024 + 256 * (p + 1), :] = core_out[256:512] + bo
    return out, res


def kernel(**inputs):
    out, _ = run(inputs, trace=False)
    return out


# revision 8
# speedup vs baseline: 1.0112x; 1.0001x over previous
"""Causal multi-head attention on 8 Trainium2 NeuronCores — v2.

Sharding: core c -> (batch g = c // 4, head-group p = c % 4, heads 4p..4p+3).
All matmuls bf16 (f32 PSUM accumulation). Causal work tiled at 128-row
k-tile granularity (fully-masked tiles skipped). Per k-tile the scores land
transposed [kpos, q] in PSUM, exp runs on the scalar engine into bf16
probs, and AV accumulates ctx^T[dk+1, q] per head (one PSUM bank per head:
a bank must never hold two open accumulation groups) with a ones column on
V putting the softmax denominators in partition 64; normalization is a DVE
reciprocal + gpsimd partition_broadcast + DVE multiply into the oproj
stationary layout. V is projected into natural [kpos, feat] layout from
resident xv. Output-projection partials are copied to bf16 SBUF, DMA'd to
DRAM per 512-row wave, ReduceScattered over each 4-core batch group, and
b_o is added on host. A budgeted interleaver paces the scores stream to
the activation engine's exp rate and fills the in-order PE with V/AV/oproj
units so no engine stalls the others.
"""

import os as os_mod
import numpy as np

B, S, D, H = 2, 2048, 1024, 16
DK = D // H
N_CORES = 8
FPC = 256  # features (head dims) per core

_CACHE = {}


def _build_nc():
    import concourse.mybir as mybir
    import concourse.tile as tile
    from concourse import bacc

    F32 = mybir.dt.float32
    BF16 = mybir.dt.bfloat16
    Exp = mybir.ActivationFunctionType.Exp

    nc = bacc.Bacc("TRN2", target_bir_lowering=False, debug=False, num_devices=8)

    cA = nc.dram_tensor("cA", [128, 2048], BF16, kind="ExternalInput")  # wq
    cB1 = nc.dram_tensor("cB1", [128, 2048], BF16, kind="ExternalInput")  # wk
    cB2 = nc.dram_tensor("cB2", [128, 4736], BF16, kind="ExternalInput")  # wv|wo|mask4|ident
    cF = nc.dram_tensor("cF", [128, 260], F32, kind="ExternalInput")  # bq|bk|bvt
    xq = nc.dram_tensor("xq", [D, S], BF16, kind="ExternalInput")
    xk = nc.dram_tensor("xk", [D, S], BF16, kind="ExternalInput")
    xv = nc.dram_tensor("xv", [D, S], BF16, kind="ExternalInput")
    out = nc.dram_tensor("out", [512, D], BF16, kind="ExternalOutput")

    no_rs = bool(os_mod.environ.get("BASS_SIM_NO_RS"))

    with tile.TileContext(nc) as tc:
        with (
            tc.tile_pool(name="consts", bufs=1) as consts,
            tc.tile_pool(name="persist", bufs=1) as persist,
            tc.tile_pool(name="xin", bufs=6) as xin,
            tc.tile_pool(name="prp", bufs=1) as prp,
            tc.tile_pool(name="small", bufs=3) as small,
            tc.tile_pool(name="oout", bufs=6) as oout,
            tc.tile_pool(name="dram", bufs=1, space="DRAM") as dram,
        ):
            # ---------------- constants ----------------
            cA_s = consts.tile([128, 2048], BF16, tag="cA", name="cA_s")
            cB1_s = consts.tile([128, 2048], BF16, tag="cB1", name="cB1_s")
            cB2_s = consts.tile([128, 4736], BF16, tag="cB2", name="cB2_s")
            cF_s = consts.tile([128, 260], F32, tag="cF", name="cF_s")
            wq_s = cA_s[:].rearrange("p (kc f) -> p kc f", kc=8)
            wk_s = cB1_s[:].rearrange("p (kc f) -> p kc f", kc=8)
            wv_s = cB2_s[:, 0:2048].rearrange("p (kc f) -> p kc f", kc=8)
            wo_s = cB2_s[:, 2048:4096].rearrange("p (c d) -> p c d", c=2)
            mask4_s = cB2_s[:, 4096:4608].rearrange("p (h x) -> p h x", h=4)
            ident_s = cB2_s[:, 4608:4736]
            bq_s = cF_s[:, 0:2]
            bk_s = cF_s[:, 2:4]
            bvt_s = cF_s[:, 4:260].rearrange("p (h x) -> p h x", h=4)

            # ---------------- persistent activations ----------------
            qT_s = persist.tile([128, 2, S], BF16, tag="qT", name="qT_s")
            kT_s = persist.tile([128, 2, S], BF16, tag="kT", name="kT_s")
            xv_s = persist.tile([128, 8, S], BF16, tag="xv", name="xv_s")
            v_s = persist.tile([128, 16, 4, 65], BF16, tag="v", name="v_s")
            ctxT_s = persist.tile([128, 2, S], BF16, tag="ctxT", name="ctxT_s")

            rs_in = [dram.tile([S // 2, D], BF16, name=f"rs_in{i}") for i in range(2)]
            rs_out = [dram.tile([256, D], BF16, name=f"rs_out{i}") for i in range(2)]

            # ones columns for the softmax denominators
            for h in range(4):
                nc.gpsimd.memset(v_s[:, :, h, 64:65], 1.0)

            # warm the Exp table at t=0 so LoadActFuncSet is off the
            # first-exp critical path
            warm = small.tile([1, 8], F32, tag="warm", bufs=1, name="warm")
            nc.vector.memset(warm[:], 0.0)
            nc.scalar.activation(out=warm[:], in_=warm[:], func=Exp)

            # ---------------- input DMA stream (SP queue order) ----------------
            xq_t, xk_t = [], []
            nc.sync.dma_start(cA_s[:, 0:256], cA[:, 0:256])
            t = xin.tile([128, S], BF16, tag="x", name="xq0")
            nc.sync.dma_start(t[:], xq[0:128, :])
            xq_t.append(t)
            nc.sync.dma_start(cA_s[:, 256:2048], cA[:, 256:2048])
            for kc in range(1, 8):
                t = xin.tile([128, S], BF16, tag="x", name=f"xq{kc}")
                nc.sync.dma_start(t[:], xq[128 * kc : 128 * (kc + 1), :])
                xq_t.append(t)
            nc.sync.dma_start(cB1_s[:], cB1.ap())
            nc.sync.dma_start(cF_s[:], cF.ap())
            for kc in range(8):
                t = xin.tile([128, S], BF16, tag="x", name=f"xk{kc}")
                nc.sync.dma_start(t[:], xk[128 * kc : 128 * (kc + 1), :])
                xk_t.append(t)
            nc.sync.dma_start(cB2_s[:], cB2.ap())
            for kc in range(8):
                nc.sync.dma_start(xv_s[:, kc, :], xv[128 * kc : 128 * (kc + 1), :])

            # ---------------- phase 1: Q/K projections ----------------
            def proj_pass(x_t, w_s, b_s, outT, psP, split_adds=False):
                ps = {}
                for pt in range(2):
                    for qb in range(4):
                        ps[(pt, qb)] = psP.tile(
                            [128, 512], F32, tag="pp", name=f"ps{pt}{qb}"
                        )
                for kc in range(7):
                    for pt in range(2):
                        for qb in range(4):
                            nc.tensor.matmul(
                                ps[(pt, qb)][:],
                                w_s[:, kc, 128 * pt : 128 * (pt + 1)],
                                x_t[kc][:, 512 * qb : 512 * (qb + 1)],
                                start=(kc == 0),
                                stop=False,
                            )
                # final contraction step: emit the bias-add right after each
                # accumulator stops so the adds pipeline with the sweep
                for i, (pt, qb) in enumerate(
                    [(pt, qb) for pt in range(2) for qb in range(4)]
                ):
                    nc.tensor.matmul(
                        ps[(pt, qb)][:],
                        w_s[:, 7, 128 * pt : 128 * (pt + 1)],
                        x_t[7][:, 512 * qb : 512 * (qb + 1)],
                        start=False,
                        stop=True,
                    )
                    nc.vector.tensor_scalar_add(
                        outT[:, pt, 512 * qb : 512 * (qb + 1)],
                        ps[(pt, qb)][:],
                        b_s[:, pt : pt + 1],
                    )

            with tc.tile_pool(name="psP", bufs=8, space="PSUM") as psP:
                proj_pass(xq_t, wq_s, bq_s, qT_s, psP)
                proj_pass(xk_t, wk_s, bk_s, kT_s, psP, split_adds=True)

            # ---------------- phase 2: attention ----------------
            pr_t = {}  # (w, ki) -> probs tile [128, 4, 512] bf16 (wave q coords)
            prm_t = {}  # (w, diag ki) -> masked probs [128, 4, 128] bf16
            ctx_t = {}  # (w, qt) -> ctx psum tile [128, 4, 128] f32
            ctxn_t = {}  # (w, qt) -> normalized ctx sbuf [128, 4, 64] bf16

            psS_pool = [None]
            psC_pool = [None]

            def sc_unit(w, ki):
                """scores + exp (+ diag mask) for (wave, ktile), one
                head-pair (2-bank) PSUM tile per exp so PE rarely waits."""
                qoff = 128 * max(ki - 4 * w, 0)
                wdt = 512 - qoff
                pr = prp.tile(
                    [128, 4, 512], BF16, tag="pr", bufs=19, name=f"pr_{w}_{ki}"
                )
                pr_t[(w, ki)] = pr
                for hp in range(2):
                    sc = psS_pool[0].tile([128, 2, 512], F32, tag="sc", name="sc")
                    for j in range(2):
                        h = 2 * hp + j
                        r, pt = 64 * (h % 2), h // 2
                        nc.tensor.matmul(
                            sc[:, j, 0:wdt],
                            kT_s[r : r + 64, pt, 128 * ki : 128 * (ki + 1)],
                            qT_s[r : r + 64, pt, 512 * w + qoff : 512 * (w + 1)],
                            start=True,
                            stop=True,
                        )
                    nc.scalar.activation(
                        out=pr[:, 2 * hp : 2 * hp + 2, qoff:512],
                        in_=sc[:, :, 0:wdt],
                        func=Exp,
                        scale=0.125,
                    )
                if ki >= 4 * w:  # diag ktile: mask the upper triangle in place
                    nc.vector.tensor_mul(
                        pr[:, :, qoff : qoff + 128],
                        pr[:, :, qoff : qoff + 128],
                        mask4_s,
                    )

            def v_unit(st):
                pv = psV_pool[0].tile([128, 256], F32, tag="pv", name="pv")
                for kc in range(8):
                    nc.tensor.matmul(
                        pv[:],
                        xv_s[:, kc, 128 * st : 128 * (st + 1)],
                        wv_s[:, kc, :],
                        start=(kc == 0),
                        stop=(kc == 7),
                    )
                nc.vector.tensor_add(
                    v_s[:, st, :, 0:64],
                    pv[:].rearrange("p (h x) -> p h x", x=64),
                    bvt_s,
                )

            def av_open(w):
                # one [65, 512] strip per PSUM bank: a bank must never hold
                # more than one open accumulation group (start=True on one
                # strip invalidates other strips' pending sums in the bank)
                for h in range(4):
                    ctx_t[(w, h)] = psC_pool[0].tile(
                        [65, 512], F32, tag="ctx", name=f"ctx{w}{h}"
                    )

            def av_mm(w, ki):
                """ctx^T[dk+1, q] += V_aug_h^T @ probs per head: the ones
                column of V_aug puts the softmax denominators in partition
                64."""
                qoff = 128 * max(ki - 4 * w, 0)
                last = 4 * w + 3
                for h in range(4):
                    nc.tensor.matmul(
                        ctx_t[(w, h)][:, qoff:512],
                        v_s[:, ki, h, :],
                        pr_t[(w, ki)][:, h, qoff:512],
                        start=(ki == 0),
                        stop=(ki == last),
                        skip_group_check=True,
                    )

            def av_fin(w, h):
                """normalize ctx^T by the denominators in partition 64."""
                r, pt = 64 * (h % 2), h // 2
                ctx = ctx_t[(w, h)]
                recip = small.tile([1, 512], F32, tag="recip", name="recip")
                nc.vector.reciprocal(recip[:], ctx[64:65, :])
                rbc = small.tile([64, 512], F32, tag="rbc", name="rbc")
                nc.gpsimd.partition_broadcast(rbc[:], recip[:])
                nc.vector.tensor_mul(
                    ctxT_s[r : r + 64, pt, 512 * w : 512 * (w + 1)],
                    ctx[0:64, :],
                    rbc[:],
                )

            def po_unit(w, u):
                qt, nb = u // 2, u % 2
                st = 4 * w + qt
                half = w // 2
                po = psC_pool[0].tile([128, 512], F32, tag="ctx", name="po")
                for fc in range(2):
                    nc.tensor.matmul(
                        po[:],
                        ctxT_s[:, fc, 128 * st : 128 * (st + 1)],
                        wo_s[:, fc, 512 * nb : 512 * (nb + 1)],
                        start=(fc == 0),
                        stop=(fc == 1),
                    )
                ot = oout.tile([128, 512], BF16, tag="ot", name="ot")
                if w == 3:  # tail wave: Act is done with exps by then
                    nc.scalar.activation(
                        out=ot[:], in_=po[:],
                        func=mybir.ActivationFunctionType.Copy,
                    )
                else:
                    nc.vector.tensor_copy(ot[:], po[:])
                nc.sync.dma_start(
                    rs_in[half][
                        512 * (w % 2) + 128 * qt : 512 * (w % 2) + 128 * (qt + 1),
                        512 * nb : 512 * (nb + 1),
                    ],
                    ot[:],
                )

            def rs_sim_out(half):
                # sim-mode stand-in for the RS result copy; reads only the
                # first wave's first two qtiles of rs_in[half]
                if no_rs:
                    nc.sync.dma_start(
                        out[256 * half : 256 * (half + 1), :], rs_in[half][0:256, :]
                    )

            def rs_unit(half):
                if not no_rs:
                    import concourse.mybir as mybir_mod

                    nc.gpsimd.collective_compute(
                        "ReduceScatter",
                        mybir_mod.AluOpType.add,
                        replica_groups=[[0, 1, 2, 3], [4, 5, 6, 7]],
                        ins=[rs_in[half].opt()],
                        outs=[rs_out[half].opt()],
                    )
                    nc.sync.dma_start(
                        out[256 * half : 256 * (half + 1), :], rs_out[half][:]
                    )

            # ---- budgeted interleave: scores paced by Act; filler units
            # (V proj, AV, oproj) sized to keep the PE exactly as busy as
            # Act's per-ktile pace, in dependency (= PSUM rotation) order.
            WAVE_ORDER = (0, 1, 2, 3)
            sc_list = [(w, ki) for w in WAVE_ORDER for ki in range(4 * w + 4)]
            sc_idx = {u: i for i, u in enumerate(sc_list)}

            def wdt_of(w, ki):
                return 512 - 128 * max(ki - 4 * w, 0)

            def act_cost(w, ki):  # us of Act work per sc unit
                return (4 * wdt_of(w, ki) * 0.833 + 2 * 185) / 1000.0

            fillers = []  # (pe_cost_us, gate_sc_index, fn)

            def F(cost, gate, fn, *a):
                fillers.append((cost, gate, lambda a=a: fn(*a)))

            GV = 13  # V units gated until a few sc units in (xv lands late)
            for st in range(16):
                F(0.85, GV, v_unit, st)
            for w in WAVE_ORDER:
                F(0.0, None, av_open, w)
                for ki in range(4 * w + 4):
                    F(
                        4 * wdt_of(w, ki) * 0.4167 / 1000.0,
                        min(sc_idx[(w, ki)] + 3, len(sc_list)),
                        av_mm, w, ki,
                    )
                for h in range(4):
                    F(0.0, None, av_fin, w, h)
                for u in range(8):
                    F(0.43, None, po_unit, w, u)
                    if w in (0, 2) and u == 3:
                        F(0.0, None, rs_sim_out, w // 2)
                if w in (1, 3):
                    F(0.0, None, rs_unit, w // 2)

            state = {"budget": 0.0, "sci": 0, "popped": 0}

            def pump(force=False, max_pop=None):
                while fillers:
                    if max_pop is not None and state["popped"] >= max_pop:
                        break
                    cost, gate, fn = fillers[0]
                    if gate is not None and state["sci"] < gate:
                        break
                    if not force and state["budget"] < cost:
                        break
                    fillers.pop(0)
                    fn()
                    state["popped"] += 1
                    state["budget"] -= cost

            with tc.tile_pool(name="psS", bufs=2, space="PSUM") as psS:
                psS_pool[0] = psS
                with tc.tile_pool(name="psV", bufs=4, space="PSUM") as psV:
                    psV_pool = [psV]
                    # phase A: emit scores, pumping only the 16 V-proj fillers
                    while state["popped"] < 16:
                        w, ki = sc_list[state["sci"]]
                        sc_unit(w, ki)
                        state["sci"] += 1
                        state["budget"] += 1.08 * act_cost(w, ki) - 0.85
                        pump(max_pop=16)
                with tc.tile_pool(name="psC", bufs=4, space="PSUM") as psC:
                    psC_pool[0] = psC
                    while state["sci"] < len(sc_list):
                        w, ki = sc_list[state["sci"]]
                        sc_unit(w, ki)
                        state["sci"] += 1
                        state["budget"] += 1.08 * act_cost(w, ki) - 0.85
                        pump()
                    pump(force=True)

    nc.compile()
    return nc


def _prep_inputs(query, key_, value, w_q, b_q, w_k, b_k, w_v, b_v, w_o, b_o):
    """Build the 8 per-core input maps (host-side sharding / re-layout)."""
    import ml_dtypes

    f32 = np.float32
    bf16 = ml_dtypes.bfloat16

    def pack_w(wT_slice):  # [1024, 256] -> [128, 8, 256] -> [128, 2048]
        return (
            np.ascontiguousarray(
                wT_slice.reshape(8, 128, FPC).transpose(1, 0, 2).reshape(128, 2048)
            ).astype(bf16)
        )

    r = np.arange(128)
    mask = (r[None, :] >= r[:, None]).astype(f32)  # [kpos, q] allowed: q >= k
    mask4 = np.tile(mask, (1, 4)).astype(bf16)  # [128, 512]
    ident = np.eye(128, dtype=f32).astype(bf16)

    wqT = np.asarray(w_q, f32).T
    wkT = np.asarray(w_k, f32).T
    wvT = np.asarray(w_v, f32).T
    woT = np.asarray(w_o, f32).T

    xT = {}
    for g in range(B):
        xT[("q", g)] = np.ascontiguousarray(np.asarray(query[g], f32).T).astype(bf16)
        xT[("k", g)] = np.ascontiguousarray(np.asarray(key_[g], f32).T).astype(bf16)
        xT[("v", g)] = np.ascontiguousarray(np.asarray(value[g], f32).T).astype(bf16)

    in_maps = []
    for c in range(N_CORES):
        g, p = c // 4, c % 4
        fsel = slice(FPC * p, FPC * (p + 1))
        woc = (
            np.ascontiguousarray(
                woT[fsel, :].reshape(2, 128, D).transpose(1, 0, 2).reshape(128, 2048)
            ).astype(bf16)
        )
        cB2_arr = np.concatenate(
            [pack_w(wvT[:, fsel]), woc, mask4, ident], axis=1
        )
        bq_c = np.asarray(b_q, f32)[fsel].reshape(2, 128).T
        bk_c = np.asarray(b_k, f32)[fsel].reshape(2, 128).T
        bvt = np.broadcast_to(np.asarray(b_v, f32)[fsel], (128, FPC))
        cF_arr = np.concatenate([bq_c, bk_c, bvt], axis=1)
        in_maps.append(
            {
                "cA": pack_w(wqT[:, fsel]),
                "cB1": pack_w(wkT[:, fsel]),
                "cB2": np.ascontiguousarray(cB2_arr),
                "cF": np.ascontiguousarray(cF_arr.astype(f32)),
                "xq": xT[("q", g)],
                "xk": xT[("k", g)],
                "xv": xT[("v", g)],
            }
        )
    return in_maps


def run(inputs, trace=False):
    from concourse.bass_utils import run_bass_kernel_spmd

    if "nc" not in _CACHE:
        _CACHE["nc"] = _build_nc()
    nc = _CACHE["nc"]
    in_maps = _prep_inputs(
        inputs["query"], inputs["key_"], inputs["value"],
        inputs["w_q"], inputs["b_q"], inputs["w_k"], inputs["b_k"],
        inputs["w_v"], inputs["b_v"], inputs["w_o"], inputs["b_o"],
    )
    res = run_bass_kernel_spmd(
        nc, in_maps, core_ids=list(range(N_CORES)), trace=trace,
    )
    bo = np.asarray(inputs["b_o"], np.float32)
    out = np.empty((B, S, D), np.float32)
    for c in range(N_CORES):
        g, p = c // 4, c % 4
        # RS half i scatters q rows [1024*i + 256*p, 1024*i + 256*(p+1))
        core_out = np.asarray(res.results[c]["out"], np.float32)
        out[g, 256 * p : 256 * (p + 1), :] = core_out[0:256] + bo
        out[g, 1024 + 256 * p : 1024 + 256 * (p + 1), :] = core_out[256:512] + bo
    return out, res


def kernel(**inputs):
    out, _ = run(inputs, trace=False)
    return out
